# revision 3
# baseline (speedup 1.0000x reference)
"""Trainium2 Bass kernel for nn_MultiHeadAttention_79534204387726.

Reference computation (B=4, S=1024, E=1024, H=16, dh=64):
    q/k/v = proj(x) = x @ Wq_w.T + Wq_b       (same Wq applied to q, k, v)
    scores = q @ k.T / 8 per head; attn = softmax(scores)
    out = (attn @ v).concat_heads @ Wo_w.T + Wo_b

Sharding (8 cores): core c -> batch b = c//2, head-group g = c%2 (8 heads,
512 features). Host sums the two partial outputs per batch and adds the
folded bias (Wo_b + Wo@Wq_b; K-bias is softmax-invariant and dropped).

v2 design (cost-model driven, TimelineSim):
  - The kernel is paced by the ACT exp stream (64 x [128,1024] exp tiles =
    66.4us, the single-engine floor).  Emission weaves ~1.65us of filler PE
    work between each scores(jt, kt) block so the PE (89us of work) stays
    busy under the exp cadence: v-proj and j-tile-2/3 projections fill the
    early pairs, AV + transposes of the previous pair and early out-proj
    passes fill the later ones.
  - scores matmuls run in fp8e4m3 DoubleRow (0.5 cycles/row): q/k are
    quantized to fp8 during the psum drain (q with bias add), staged to
    DRAM, and reloaded as [32, head, dsub, S] (the 128->32 partition fold
    needs a DMA round trip).  Those DMAs ride the ACT DGE queue so they
    don't queue behind the input loads on SP.
  - The first quarter of the q/k projection contraction (e-tiles 0-1) also
    runs in fp8 DoubleRow from host-quantized x8/wq8.  Host-measured
    rel_err for fp8 scores inputs + 25% fp8 proj: 0.0157 (gate 2e-2).
  - AV is flipped to out[q, d]: lhsT = exp(scores^T) [k, q] chunk, rhs = V
    [k, 65] per head (64 dims + ones column -> denominator lands in psum
    col 64 per q row): 65 free-cols/instruction instead of 512.  Two heads
    pack into one psum bank (second head's first write exploits the lazy
    2KB zero-region).
  - softmax normalize runs on gpsimd (normalize_recip: divide + fp16 cast).
  - C comes out as [q, d]; a PE transpose (identity matmul) restores the
    [feature, seq] layout the out-projection needs as lhsT.
  - out-proj is split: ft0+ft1 partials during pair 2, += ft2 during pair
    3 (fp16 SBUF staging, identity-matmul re-accumulation), ft3 + final
    add in the tail so only ~1.4us/st of PE work remains after the last
    exp.
"""

import numpy as np
import ml_dtypes

B, S, E, H = 4, 1024, 1024, 16
NCORES = 8
EH = E // 2        # 512 features per head-group
NHG = H // 2       # 8 heads per group
DH = E // H        # 64
P = 128
NE = E // P        # 8 e-tiles over full E
NF8 = 2            # e-tiles 0-1 of the q/k projection contract in fp8 DR
NE16 = NE - NF8    # e-tiles 2-7 in fp16
NJ = EH // P       # 4 j-tiles over the group's 512 features
NQ = S // 512      # 2 query/sequence chunks of 512
NST = S // P       # 8 sequence tiles of 128
F16 = np.float16
F8 = ml_dtypes.float8_e4m3fn

_CACHE = {}


def _build_program(reps=1):
    import concourse.tile as tile
    from concourse import bacc, mybir
    from concourse.masks import make_identity
    from contextlib import ExitStack

    f32 = mybir.dt.float32
    f16 = mybir.dt.float16
    f8 = mybir.dt.float8e4
    AF = mybir.ActivationFunctionType
    DR = mybir.MatmulPerfMode.DoubleRow

    nc = bacc.Bacc(
        "TRN2",
        target_bir_lowering=False,
        debug=False,
        num_devices=NCORES,
    )

    # fp16 x for q/k carries only e-tiles 2-7; tiles 0-1 come as fp8
    xq_t = nc.dram_tensor("xq_t", [NE16 * P, S], f16, kind="ExternalInput")
    xk_t = nc.dram_tensor("xk_t", [NE16 * P, S], f16, kind="ExternalInput")
    xv_t = nc.dram_tensor("xv_t", [E, S], f16, kind="ExternalInput")
    xq8_t = nc.dram_tensor("xq8_t", [NF8 * P, S], f8, kind="ExternalInput")
    xk8_t = nc.dram_tensor("xk8_t", [NF8 * P, S], f8, kind="ExternalInput")
    wq_t = nc.dram_tensor("wq_t", [E, EH], f16, kind="ExternalInput")
    wq8_t = nc.dram_tensor("wq8_t", [NF8 * P, EH], f8, kind="ExternalInput")
    wo_t = nc.dram_tensor("wo_t", [EH, E], f16, kind="ExternalInput")
    bq = nc.dram_tensor("bq", [P, NJ], f32, kind="ExternalInput")
    out_d = nc.dram_tensor("out_partial", [S, E], f16, kind="ExternalOutput")
    # DRAM staging for the fp8 partition regroup (128 -> 32x4)
    q_stage = nc.dram_tensor("q_stage", [NJ, P, S], f8, kind="Internal")
    k_stage = nc.dram_tensor("k_stage", [NJ, P, S], f8, kind="Internal")

    vw = DH + 1        # per-head V columns incl. the ones column

    with tile.TileContext(nc) as tc, ExitStack() as ctx:
        const = ctx.enter_context(tc.tile_pool(name="const", bufs=1))
        pt_pool = ctx.enter_context(tc.tile_pool(name="pt", bufs=32))
        cu_pool = ctx.enter_context(tc.tile_pool(name="cu", bufs=6))
        cq_pool = ctx.enter_context(tc.tile_pool(name="cq", bufs=6))
        outp = ctx.enter_context(tc.tile_pool(name="outp", bufs=6))
        ps_s = ctx.enter_context(tc.tile_pool(name="ps_s", bufs=2, space="PSUM"))
        ps_o = ctx.enter_context(tc.tile_pool(name="ps_o", bufs=2, space="PSUM"))
        ps_t = ctx.enter_context(tc.tile_pool(name="ps_t", bufs=1, space="PSUM"))
        # junk ramp-keeper matmuls write here: they keep the PE engine busy
        # through known feed gaps so the p-state never drops (matmul cost
        # is priced at dispatch with pe_busy_start reset on any engine
        # idle); single buf -> junk serializes on itself only
        ps_j = ctx.enter_context(tc.tile_pool(name="ps_j", bufs=1, space="PSUM"))

        # ---- resident SBUF tensors ----
        wq_sb = const.tile([P, NE, EH], f16)     # full W (v-proj + qk 2-7)
        wq8_sb = const.tile([P, NF8, EH], f8)
        wo_sb = const.tile([P, NJ, E], f16)      # [p, f-tile, o]
        bq_sb = const.tile([P, NJ], f32)
        junk = const.tile([P, 512], f16)
        ident = const.tile([P, P], f16)
        xq_sb = const.tile([P, NE16, S], f16)
        xk_sb = const.tile([P, NE16, S], f16)
        xq8_sb = const.tile([P, NF8, S], f8)
        xk8_sb = const.tile([P, NF8, S], f8)
        xv_sb = const.tile([P, NE, S], f16)
        # fp8 q/k staging (proj drain output, pre-regroup) [p=j, jt, s]
        q8_sb = const.tile([P, NJ - 1, S], f8)
        k8_sb = const.tile([P, NJ - 1, S], f8)
        # pair 0 (j-tile 0) keeps q/k in fp16 and runs fp16 scores straight
        # from the drain: no DRAM regroup round-trip on the critical path
        qt0_sb = const.tile([P, S], f16)
        kt0_sb = const.tile([P, S], f16)
        # DR-layout q/k [32, jt-1, head, dsub, s] (j-tiles 1..3 only;
        # pair 0 runs fp16)
        q8dr = const.tile([32, NJ - 1, 2, 2, S], f8)
        k8dr = const.tile([32, NJ - 1, 2, 2, S], f8)
        # V tiles [key-tile][p=k, 8 heads x (dh + ones col)]
        v_sb = [const.tile([P, NHG * vw], f16, tag=f"v{st}", name=f"v{st}")
                for st in range(NST)]
        # transposed attention output C^T [p=f, f-tile, s] fp16
        ct_sb = const.tile([P, NJ, S], f16)
        # out-proj partial accumulator [p=s, st, o] fp16
        oa_sb = const.tile([P, NST, S], f16)

        nc.sync.dma_start(out=bq_sb[:, :], in_=bq[:, :])
        nc.vector.memset(junk, 0.0)
        make_identity(nc, ident)
        wq_r = wq_t.rearrange("(t p) j -> p t j", p=P)
        wq8_r = wq8_t.rearrange("(t p) j -> p t j", p=P)
        xk_r = xk_t.rearrange("(t p) s -> p t s", p=P)
        xq_r = xq_t.rearrange("(t p) s -> p t s", p=P)
        xk8_r = xk8_t.rearrange("(t p) s -> p t s", p=P)
        xq8_r = xq8_t.rearrange("(t p) s -> p t s", p=P)
        xv_r = xv_t.rearrange("(t p) s -> p t s", p=P)
        wo_r = wo_t.rearrange("(t p) o -> p t o", p=P)
        # input DMA order tuned so the k/q projections can stream in
        # two-tile waves: fp8 parts first (tiny), then alternating
        # (xk, wq) two-tile transfers, then xq; wq e-tiles 0-1 (fp16,
        # only v-proj needs them), xv and wo stream last -- pair-0 scores
        # run fp16 straight from the drain so nothing critical queues
        # behind them in the serialized DMA-engine FIFO
        nc.sync.dma_start(out=xk8_sb[...], in_=xk8_r)
        nc.sync.dma_start(out=wq8_sb[...], in_=wq8_r)
        nc.sync.dma_start(out=xq8_sb[...], in_=xq8_r)
        for t in range(0, NE16, 2):
            nc.sync.dma_start(out=xk_sb[:, t:t + 2, :], in_=xk_r[:, t:t + 2, :])
            nc.sync.dma_start(out=wq_sb[:, t + NF8:t + NF8 + 2, :],
                              in_=wq_r[:, t + NF8:t + NF8 + 2, :])
        for t in range(0, NE16, 2):
            nc.sync.dma_start(out=xq_sb[:, t:t + 2, :], in_=xq_r[:, t:t + 2, :])
        nc.sync.dma_start(out=wq_sb[:, 0:NF8, :], in_=wq_r[:, 0:NF8, :])
        for h in range(2):
            sl = slice(h * 4, (h + 1) * 4)
            nc.sync.dma_start(out=xv_sb[:, sl, :], in_=xv_r[:, sl, :])
        nc.sync.dma_start(out=wo_sb[:, :, :], in_=wo_r)

        def body():
            def jmm(n, width=512):
                # junk ramp-keeper matmuls: ~213ns each of always-ready PE
                # work (they only read the zeroed junk tile)
                for _ in range(n):
                    jp = ps_j.tile([P, 512], f32, tag="junk", name="jp")
                    nc.tensor.matmul(jp[:, 0:width], lhsT=junk[:, 0:128],
                                     rhs=junk[:, 0:width],
                                     start=True, stop=True)

            # warm-up: bridge from t=0 until the first k-proj inputs land
            jmm(16)

            for st in range(NST):
                vh = v_sb[st].rearrange("p (h c) -> p h c", c=vw)
                nc.vector.memset(vh[:, :, DH], 1.0)

            # psum slots for projection-phase matmul groups: ps_o always;
            # ps_s only while it isn't busy with scores (head phase)
            rr_state = [0]

            def rr_psum(head=False):
                if head:
                    i = rr_state[0] % 4
                    rr_state[0] += 1
                    if i < 2:
                        t = ps_s.tile([P, S], f32, tag="ps_s", name="ps")
                        return t[:, 0:512]
                return ps_o.tile([P, 512], f32, tag="ps_o", name="ps")

            def proj_qk_group(jt, qc, x16, x8, dst8, bias, head=False):
                # one (jt, qc) projection group: 1 fp8-DR matmul over
                # e-tiles 0-1 + 6 fp16 matmuls over e-tiles 2-7
                ps = rr_psum(head)
                nc.tensor.matmul(
                    ps,
                    lhsT=wq8_sb[:, :, jt * P:(jt + 1) * P],
                    rhs=x8[:, :, qc * 512:(qc + 1) * 512],
                    start=True, stop=False,
                    perf_mode=DR,
                )
                for t in range(NF8, NE):
                    nc.tensor.matmul(
                        ps,
                        lhsT=wq_sb[:, t, jt * P:(jt + 1) * P],
                        rhs=x16[:, t - NF8, qc * 512:(qc + 1) * 512],
                        start=False,
                        stop=(t == NE - 1),
                    )
                d = dst8[:, jt - 1, qc * 512:(qc + 1) * 512]
                if bias is not None:
                    nc.vector.tensor_scalar_add(d, ps, bias[:, jt:jt + 1])
                else:
                    nc.vector.tensor_copy(d, ps)

            def regroup(jt, src8, stage, dstdr):
                # SBUF -> DRAM -> SBUF partition fold 128 -> 32x(2 head,
                # 2 dsub); rides the gpsimd SWDGE queue so neither the SP
                # input stream nor the ACT exp queue serializes against it;
                # src8 holds j-tiles 1..3 at jt-1
                nc.gpsimd.dma_start(out=stage[jt], in_=src8[:, jt - 1, :])
                nc.gpsimd.dma_start(
                    out=dstdr[:, jt - 1, :, :, :],
                    in_=stage[jt].rearrange("(h d q) s -> q h d s", h=2, d=2),
                )

            def proj_v(st):
                ps = rr_psum()
                for t in range(NE):
                    nc.tensor.matmul(
                        ps,
                        lhsT=xv_sb[:, t, st * P:(st + 1) * P],
                        rhs=wq_sb[:, t, :],
                        start=(t == 0),
                        stop=(t == NE - 1),
                    )
                vh = v_sb[st].rearrange("p (h c) -> p h c", c=vw)
                nc.vector.tensor_copy(
                    vh[:, :, 0:DH],
                    ps.rearrange("p (h d) -> p h d", d=DH))

            pt_pairs = [[[], []] for _ in range(NJ)]

            def scores_exp_kt(jt, kt):
                # fp8 DoubleRow scores for the head pair of j-tile jt at
                # key-tile kt: per hh one [128, 1024] psum tile (2 banks)
                # covering both q-chunks; 2 DR matmuls fill it; exp is one
                # wide ACT op
                for hh in range(2):
                    pss = ps_s.tile([P, S], f32, tag="ps_s", name=f"pss{hh}")
                    pt = pt_pool.tile([P, S], f16, tag="pt", name=f"pt{hh}")
                    pt_pairs[jt][hh].append(pt)
                    bp = hh * DH
                    for qc in range(NQ):
                        if jt == 0:
                            nc.tensor.matmul(
                                pss[:, qc * 512:(qc + 1) * 512],
                                lhsT=kt0_sb[bp:bp + DH, kt * P:(kt + 1) * P],
                                rhs=qt0_sb[bp:bp + DH,
                                           qc * 512:(qc + 1) * 512],
                                start=True, stop=True,
                            )
                        else:
                            nc.tensor.matmul(
                                pss[:, qc * 512:(qc + 1) * 512],
                                lhsT=k8dr[:, jt - 1, hh, :,
                                          kt * P:(kt + 1) * P],
                                rhs=q8dr[:, jt - 1, hh, :,
                                         qc * 512:(qc + 1) * 512],
                                start=True, stop=True,
                                perf_mode=DR,
                            )
                    nc.scalar.activation(
                        out=pt, in_=pss, func=AF.Exp, scale=0.125,
                    )

            cq_store = {}

            def av_pair(jt, st):
                # AV for both heads of pair jt at query tile st, packed
                # into one psum bank: head hh occupies cols [hh*65,
                # hh*65+65) (64 dims + denominator from the V ones column)
                pts = pt_pairs[jt]
                po = ps_o.tile([P, 512], f32, tag="ps_o", name="po")
                for kt in range(NE):
                    for hh in range(2):
                        h = 2 * jt + hh
                        nc.tensor.matmul(
                            po[:, hh * vw:(hh + 1) * vw],
                            lhsT=pts[hh][kt][:, st * P:(st + 1) * P],
                            rhs=v_sb[kt][:, h * vw:(h + 1) * vw],
                            start=(kt == 0 and hh == 0),
                            stop=(kt == NE - 1 and hh == 1),
                            skip_group_check=True,
                        )
                cu = cu_pool.tile([P, 2 * vw], f32, tag="cu", name="cu")
                nc.vector.tensor_copy(cu, po[:, 0:2 * vw])
                cq = cq_pool.tile([P, 2, DH], f16, tag="cq", name="cq")
                for hh in range(2):
                    nc.gpsimd.normalize_recip(
                        cq[:, hh, :],
                        cu[:, hh * vw:hh * vw + DH],
                        cu[:, hh * vw + DH:(hh + 1) * vw],
                    )
                cq_store[(jt, st)] = cq

            def transpose_pair(jt, st):
                # [128 q, 64 d] per head -> psum [64 d, 128 q] stacked pair
                cq = cq_store.pop((jt, st))
                pst = ps_t.tile([P, P], f16, tag="ps_t", name="pst")
                for hh in range(2):
                    nc.tensor.matmul(
                        pst[hh * DH:(hh + 1) * DH, :],
                        lhsT=cq[:, hh, :],
                        rhs=ident,
                        is_transpose=True,
                        tile_position=(0, hh * DH),
                        skip_group_check=True,
                    )
                dst = ct_sb[:, jt, st * P:(st + 1) * P]
                if jt == NJ - 1:
                    # tail pair: ACT is free once the exp stream ends
                    nc.scalar.copy(dst, pst)
                else:
                    nc.vector.tensor_copy(dst, pst)

            def av_step(p, st):
                av_pair(p, st)
                if st >= 2:
                    transpose_pair(p, st - 2)
                if st == NST - 1:
                    transpose_pair(p, st - 1)
                    transpose_pair(p, st)

            def op_a(st):
                # out-proj pass A: ft0+ft1+ft2 partial -> oa (fp16 staging)
                for oc in range(NQ):
                    ps = ps_o.tile([P, 512], f32, tag="ps_o", name="ps")
                    for ft in range(3):
                        nc.tensor.matmul(
                            ps,
                            lhsT=ct_sb[:, ft, st * P:(st + 1) * P],
                            rhs=wo_sb[:, ft, oc * 512:(oc + 1) * 512],
                            start=(ft == 0),
                            stop=(ft == 2),
                        )
                    nc.vector.tensor_copy(
                        oa_sb[:, st, oc * 512:(oc + 1) * 512], ps)

            def op_b(st, split_dma=False):
                # out-proj pass B (tail): ft3 + oa -> out, per-oc tiles and
                # stores (drains split across DVE and gpsimd)
                for oc in range(NQ):
                    sl = slice(oc * 512, (oc + 1) * 512)
                    ot = outp.tile([P, 512], f16, tag="ot", name="ot")
                    pst = ps_s.tile([P, S], f32, tag="ps_s", name="ps")
                    ps = pst[:, 0:512]
                    nc.tensor.matmul(
                        ps,
                        lhsT=ct_sb[:, 3, st * P:(st + 1) * P],
                        rhs=wo_sb[:, 3, sl],
                        start=True, stop=False,
                    )
                    nc.tensor.matmul(
                        ps,
                        lhsT=ident,
                        rhs=oa_sb[:, st, sl],
                        start=False, stop=True,
                    )
                    if oc == 0:
                        nc.vector.tensor_copy(ot, ps)
                    else:
                        # ACT is idle in the tail (exp stream done)
                        nc.scalar.copy(ot, ps)
                    nc.sync.dma_start(
                        out=out_d[st * P:(st + 1) * P, sl],
                        in_=ot,
                    )

            # ---- emission ----
            # head: k-proj then q-proj for j-tiles 0,1, tile-major with
            # junk bridges sized to the DMA arrival cadence (per-tile
            # transfer ~0.7us unlocks 4 matmuls ~0.85us; junk fills the
            # start-up slack).  JH tunables were set from the sim trace.
            JH = [6, 0, 0, 0, 0, 0]
            kq01 = [(jt, qc) for jt in (0, 1) for qc in range(NQ)]
            pss_k = {g: rr_psum(True) for g in kq01}
            for g in kq01:
                nc.tensor.matmul(
                    pss_k[g],
                    lhsT=wq8_sb[:, :, g[0] * P:(g[0] + 1) * P],
                    rhs=xk8_sb[:, :, g[1] * 512:(g[1] + 1) * 512],
                    start=True, stop=False, perf_mode=DR)
            for t in range(NF8, NE):
                jmm(JH[t - NF8])
                for jt, qc in kq01:
                    nc.tensor.matmul(
                        pss_k[(jt, qc)],
                        lhsT=wq_sb[:, t, jt * P:(jt + 1) * P],
                        rhs=xk_sb[:, t - NF8, qc * 512:(qc + 1) * 512],
                        start=False, stop=(t == NE - 1))
            for jt, qc in kq01:
                if jt == 0:
                    nc.vector.tensor_copy(
                        kt0_sb[:, qc * 512:(qc + 1) * 512], pss_k[(jt, qc)])
                else:
                    nc.vector.tensor_copy(
                        k8_sb[:, jt - 1, qc * 512:(qc + 1) * 512],
                        pss_k[(jt, qc)])
            regroup(1, k8_sb, k_stage, k8dr)
            # q-proj: j-tile 0 alone first so its drain (the exp-stream
            # gate) lands as early as possible after the last xq tile;
            # j-tile 1 follows (its regroup has a whole pair of slack)
            for jts in ((0,), (1,)):
                qg = [(jt, qc) for jt in jts for qc in range(NQ)]
                pss_q = {g: rr_psum(True) for g in qg}
                for g in qg:
                    nc.tensor.matmul(
                        pss_q[g],
                        lhsT=wq8_sb[:, :, g[0] * P:(g[0] + 1) * P],
                        rhs=xq8_sb[:, :, g[1] * 512:(g[1] + 1) * 512],
                        start=True, stop=False, perf_mode=DR)
                for t in range(NF8, NE):
                    for jt, qc in qg:
                        nc.tensor.matmul(
                            pss_q[(jt, qc)],
                            lhsT=wq_sb[:, t, jt * P:(jt + 1) * P],
                            rhs=xq_sb[:, t - NF8, qc * 512:(qc + 1) * 512],
                            start=False, stop=(t == NE - 1))
                for jt, qc in qg:
                    dst = (qt0_sb[:, qc * 512:(qc + 1) * 512] if jt == 0 else
                           q8_sb[:, jt - 1, qc * 512:(qc + 1) * 512])
                    nc.vector.tensor_scalar_add(dst, pss_q[(jt, qc)],
                                                bq_sb[:, jt:jt + 1])
            regroup(1, q8_sb, q_stage, q8dr)

            # filler thunks woven between scores blocks
            def kq(jt, qc, x16, x8, dst8, bias):
                return lambda: proj_qk_group(jt, qc, x16, x8, dst8, bias)

            def rg(jt, src8, stage, dstdr):
                return lambda: regroup(jt, src8, stage, dstdr)

            def av(p, st):
                return lambda: av_step(p, st)

            def vp(st):
                return lambda: proj_v(st)

            def jk(n):
                return lambda: jmm(n)

            fillers = {
                # pair 0: j-tile-2 projections early (regroups well before
                # pair 2), then the first v-projections
                0: [[kq(2, 0, xk_sb, xk8_sb, k8_sb, None), jk(2)],
                    [kq(2, 1, xk_sb, xk8_sb, k8_sb, None),
                     rg(2, k8_sb, k_stage, k8dr), jk(2)],
                    [kq(2, 0, xq_sb, xq8_sb, q8_sb, bq_sb), jk(2)],
                    [kq(2, 1, xq_sb, xq8_sb, q8_sb, bq_sb),
                     rg(2, q8_sb, q_stage, q8dr), jk(2)],
                    [vp(0)], [vp(1)], [vp(2)], [vp(3)]],
                # pair 1: rest of v-proj, then AV(0)
                1: [[vp(4)], [vp(5)], [vp(6)], [vp(7)],
                    [av(0, 0), av(0, 1), jk(2)],
                    [av(0, 2), av(0, 3), jk(2)],
                    [av(0, 4), av(0, 5), jk(2)],
                    [av(0, 6), av(0, 7), jk(2)]],
                # pair 2: j-tile-3 projections early + AV(1)
                2: [[kq(3, 0, xk_sb, xk8_sb, k8_sb, None), av(1, 0)],
                    [kq(3, 1, xk_sb, xk8_sb, k8_sb, None),
                     rg(3, k8_sb, k_stage, k8dr), av(1, 1)],
                    [kq(3, 0, xq_sb, xq8_sb, q8_sb, bq_sb), av(1, 2)],
                    [kq(3, 1, xq_sb, xq8_sb, q8_sb, bq_sb),
                     rg(3, q8_sb, q_stage, q8dr), av(1, 3)],
                    [av(1, 4), jk(2)],
                    [av(1, 5), jk(2)],
                    [av(1, 6), jk(2)],
                    [av(1, 7), jk(2)]],
                # pair 3: AV(2) first (pt ring), then out-proj pass A
                3: [[av(2, 0), av(2, 1), jk(2)],
                    [av(2, 2), av(2, 3), jk(2)],
                    [av(2, 4), av(2, 5), jk(2)],
                    [av(2, 6), av(2, 7), jk(2)],
                    [lambda: op_a(0), lambda: op_a(1)],
                    [lambda: op_a(2), lambda: op_a(3)],
                    [lambda: op_a(4), lambda: op_a(5)],
                    [lambda: op_a(6), lambda: op_a(7)]],
            }
            for p in range(NJ):
                for kt in range(NE):
                    scores_exp_kt(p, kt)
                    for th in fillers[p][kt]:
                        th()

            # tail: AV(3) -> transpose -> out-proj pass B chase (lag 2)
            for st in range(NST):
                av_pair(3, st)
                if st >= 1:
                    transpose_pair(3, st - 1)
                if st >= 2:
                    op_b(st - 2)
            transpose_pair(3, NST - 1)
            for st in range(NST - 2, NST):
                op_b(st)

        for _ in range(reps):
            body()

    nc.finalize()
    return nc


def _get_nc(reps=1):
    key = ("nc", reps)
    if key not in _CACHE:
        _CACHE[key] = _build_program(reps)
    return _CACHE[key]


def make_in_maps(queries, keys, values, Wq_w, Wq_b, Wo_w, Wo_b):
    in_maps = []
    cut = NF8 * P
    for c in range(NCORES):
        b, g = c // 2, c % 2
        js = slice(g * EH, (g + 1) * EH)
        qT = np.ascontiguousarray(queries[b].T)
        kT = np.ascontiguousarray(keys[b].T)
        wT = np.ascontiguousarray(Wq_w[js, :].T)
        in_maps.append({
            "xq_t": qT[cut:].astype(F16),
            "xk_t": kT[cut:].astype(F16),
            "xv_t": np.ascontiguousarray(values[b].T).astype(F16),
            "xq8_t": qT[:cut].astype(F8),
            "xk8_t": kT[:cut].astype(F8),
            "wq_t": wT.astype(F16),
            "wq8_t": wT[:cut].astype(F8),
            "wo_t": np.ascontiguousarray(Wo_w[:, js].T).astype(F16),
            "bq": np.ascontiguousarray(Wq_b[js].reshape(NJ, P).T),
        })
    return in_maps


def assemble_output(results, Wq_b, Wo_w, Wo_b):
    bias_total = (Wo_w @ Wq_b + Wo_b).astype(np.float32)
    out = np.empty((B, S, E), np.float32)
    for b in range(B):
        out[b] = (results[2 * b]["out_partial"].astype(np.float32)
                  + results[2 * b + 1]["out_partial"].astype(np.float32))
    out += bias_total
    return out


def kernel(queries, keys, values, Wq_w, Wq_b, Wo_w, Wo_b, num_heads):
    from concourse.bass_utils import run_bass_kernel_spmd

    queries = np.asarray(queries, np.float32)
    keys = np.asarray(keys, np.float32)
    values = np.asarray(values, np.float32)
    Wq_w = np.asarray(Wq_w, np.float32)
    Wq_b = np.asarray(Wq_b, np.float32)
    Wo_w = np.asarray(Wo_w, np.float32)
    Wo_b = np.asarray(Wo_b, np.float32)
    assert int(num_heads) == H

    nc = _get_nc()
    in_maps = make_in_maps(queries, keys, values, Wq_w, Wq_b, Wo_w, Wo_b)
    res = run_bass_kernel_spmd(nc, in_maps, core_ids=list(range(NCORES)))
    _CACHE["last_results"] = res
    return assemble_output(res.results, Wq_b, Wo_w, Wo_b)


# revision 4
# speedup vs baseline: 1.0551x; 1.0551x over previous
"""Trainium2 Bass kernel for nn_MultiHeadAttention_79534204387726.

Reference computation (B=4, S=1024, E=1024, H=16, dh=64):
    q/k/v = proj(x) = x @ Wq_w.T + Wq_b       (same Wq applied to q, k, v)
    scores = q @ k.T / 8 per head; attn = softmax(scores)
    out = (attn @ v).concat_heads @ Wo_w.T + Wo_b

Sharding (8 cores): core c -> batch b = c//2, head-group g = c%2 (8 heads,
512 features). Host sums the two partial outputs per batch and adds the
folded bias (Wo_b + Wo@Wq_b; K-bias is softmax-invariant and dropped).

v2 design (cost-model driven, TimelineSim):
  - The kernel is paced by the ACT exp stream (64 x [128,1024] exp tiles =
    66.4us, the single-engine floor).  Emission weaves ~1.65us of filler PE
    work between each scores(jt, kt) block so the PE (89us of work) stays
    busy under the exp cadence: v-proj and j-tile-2/3 projections fill the
    early pairs, AV + transposes of the previous pair and early out-proj
    passes fill the later ones.
  - scores matmuls run in fp8e4m3 DoubleRow (0.5 cycles/row): q/k are
    quantized to fp8 during the psum drain (q with bias add), staged to
    DRAM, and reloaded as [32, head, dsub, S] (the 128->32 partition fold
    needs a DMA round trip).  Those DMAs ride the ACT DGE queue so they
    don't queue behind the input loads on SP.
  - The first half of the q/k projection contraction (e-tiles 0-3) also
    runs in fp8 DoubleRow from host-quantized x8/wq8.  Measured end-to-end
    rel_err 0.0170 (gate 2e-2; pair 0 stays fp16 which also skips the
    regroup round-trip on the exp-stream critical path).
  - AV is flipped to out[q, d]: lhsT = exp(scores^T) [k, q] chunk, rhs = V
    [k, 65] per head (64 dims + ones column -> denominator lands in psum
    col 64 per q row): 65 free-cols/instruction instead of 512.  Two heads
    pack into one psum bank (second head's first write exploits the lazy
    2KB zero-region).
  - softmax normalize runs on gpsimd (normalize_recip: divide + fp16 cast).
  - C comes out as [q, d]; a PE transpose (identity matmul) restores the
    [feature, seq] layout the out-projection needs as lhsT.
  - out-proj is split: ft0+ft1 partials during pair 2, += ft2 during pair
    3 (fp16 SBUF staging, identity-matmul re-accumulation), ft3 + final
    add in the tail so only ~1.4us/st of PE work remains after the last
    exp.
"""

import numpy as np
import ml_dtypes

B, S, E, H = 4, 1024, 1024, 16
NCORES = 8
EH = E // 2        # 512 features per head-group
NHG = H // 2       # 8 heads per group
DH = E // H        # 64
P = 128
NE = E // P        # 8 e-tiles over full E
NF8 = 4            # e-tiles 0-3 of the q/k projection contract in fp8 DR
NE16 = NE - NF8    # e-tiles 2-7 in fp16
NJ = EH // P       # 4 j-tiles over the group's 512 features
NQ = S // 512      # 2 query/sequence chunks of 512
NST = S // P       # 8 sequence tiles of 128
F16 = np.float16
F8 = ml_dtypes.float8_e4m3fn

_CACHE = {}


def _build_program(reps=1):
    import concourse.tile as tile
    from concourse import bacc, mybir
    from concourse.masks import make_identity
    from contextlib import ExitStack

    f32 = mybir.dt.float32
    f16 = mybir.dt.float16
    f8 = mybir.dt.float8e4
    AF = mybir.ActivationFunctionType
    DR = mybir.MatmulPerfMode.DoubleRow

    nc = bacc.Bacc(
        "TRN2",
        target_bir_lowering=False,
        debug=False,
        num_devices=NCORES,
    )

    # fp16 x for q/k carries only e-tiles 2-7; tiles 0-1 come as fp8
    xq_t = nc.dram_tensor("xq_t", [NE16 * P, S], f16, kind="ExternalInput")
    xk_t = nc.dram_tensor("xk_t", [NE16 * P, S], f16, kind="ExternalInput")
    xv_t = nc.dram_tensor("xv_t", [E, S], f16, kind="ExternalInput")
    xq8_t = nc.dram_tensor("xq8_t", [NF8 * P, S], f8, kind="ExternalInput")
    xk8_t = nc.dram_tensor("xk8_t", [NF8 * P, S], f8, kind="ExternalInput")
    wq_t = nc.dram_tensor("wq_t", [E, EH], f16, kind="ExternalInput")
    wq8_t = nc.dram_tensor("wq8_t", [NF8 * P, EH], f8, kind="ExternalInput")
    wo_t = nc.dram_tensor("wo_t", [EH, E], f16, kind="ExternalInput")
    bq = nc.dram_tensor("bq", [P, NJ], f32, kind="ExternalInput")
    out_d = nc.dram_tensor("out_partial", [S, E], f16, kind="ExternalOutput")
    # DRAM staging for the fp8 partition regroup (128 -> 32x4)
    q_stage = nc.dram_tensor("q_stage", [NJ, P, S], f8, kind="Internal")
    k_stage = nc.dram_tensor("k_stage", [NJ, P, S], f8, kind="Internal")

    vw = DH + 1        # per-head V columns incl. the ones column

    with tile.TileContext(nc) as tc, ExitStack() as ctx:
        const = ctx.enter_context(tc.tile_pool(name="const", bufs=1))
        pt_pool = ctx.enter_context(tc.tile_pool(name="pt", bufs=32))
        cu_pool = ctx.enter_context(tc.tile_pool(name="cu", bufs=6))
        cq_pool = ctx.enter_context(tc.tile_pool(name="cq", bufs=6))
        outp = ctx.enter_context(tc.tile_pool(name="outp", bufs=6))
        ps_s = ctx.enter_context(tc.tile_pool(name="ps_s", bufs=2, space="PSUM"))
        ps_o = ctx.enter_context(tc.tile_pool(name="ps_o", bufs=2, space="PSUM"))
        ps_t = ctx.enter_context(tc.tile_pool(name="ps_t", bufs=1, space="PSUM"))
        # junk ramp-keeper matmuls write here: they keep the PE engine busy
        # through known feed gaps so the p-state never drops (matmul cost
        # is priced at dispatch with pe_busy_start reset on any engine
        # idle); single buf -> junk serializes on itself only
        ps_j = ctx.enter_context(tc.tile_pool(name="ps_j", bufs=1, space="PSUM"))

        # ---- resident SBUF tensors ----
        wq_sb = const.tile([P, NE, EH], f16)     # full W (v-proj + qk 2-7)
        wq8_sb = const.tile([P, NF8, EH], f8)
        wo_sb = const.tile([P, NJ, E], f16)      # [p, f-tile, o]
        bq_sb = const.tile([P, NJ], f32)
        junk = const.tile([P, 512], f16)
        ident = const.tile([P, P], f16)
        xq_sb = const.tile([P, NE16, S], f16)
        xk_sb = const.tile([P, NE16, S], f16)
        xq8_sb = const.tile([P, NF8, S], f8)
        xk8_sb = const.tile([P, NF8, S], f8)
        xv_sb = const.tile([P, NE, S], f16)
        # fp8 q/k staging (proj drain output, pre-regroup) [p=j, jt, s]
        q8_sb = const.tile([P, NJ - 1, S], f8)
        k8_sb = const.tile([P, NJ - 1, S], f8)
        # pair 0 (j-tile 0) keeps q/k in fp16 and runs fp16 scores straight
        # from the drain: no DRAM regroup round-trip on the critical path
        qt0_sb = const.tile([P, S], f16)
        kt0_sb = const.tile([P, S], f16)
        # DR-layout q/k [32, jt-1, head, dsub, s] (j-tiles 1..3 only;
        # pair 0 runs fp16)
        q8dr = const.tile([32, NJ - 1, 2, 2, S], f8)
        k8dr = const.tile([32, NJ - 1, 2, 2, S], f8)
        # V tiles [key-tile][p=k, 8 heads x (dh + ones col)]
        v_sb = [const.tile([P, NHG * vw], f16, tag=f"v{st}", name=f"v{st}")
                for st in range(NST)]
        # transposed attention output C^T [p=f, f-tile, s] fp16
        ct_sb = const.tile([P, NJ, S], f16)
        # out-proj partial accumulator [p=s, st, o] fp16
        oa_sb = const.tile([P, NST, S], f16)

        nc.sync.dma_start(out=bq_sb[:, :], in_=bq[:, :])
        nc.vector.memset(junk, 0.0)
        make_identity(nc, ident)
        wq_r = wq_t.rearrange("(t p) j -> p t j", p=P)
        wq8_r = wq8_t.rearrange("(t p) j -> p t j", p=P)
        xk_r = xk_t.rearrange("(t p) s -> p t s", p=P)
        xq_r = xq_t.rearrange("(t p) s -> p t s", p=P)
        xk8_r = xk8_t.rearrange("(t p) s -> p t s", p=P)
        xq8_r = xq8_t.rearrange("(t p) s -> p t s", p=P)
        xv_r = xv_t.rearrange("(t p) s -> p t s", p=P)
        wo_r = wo_t.rearrange("(t p) o -> p t o", p=P)
        # input DMA order tuned so the k/q projections can stream in
        # two-tile waves: fp8 parts first (tiny), then alternating
        # (xk, wq) two-tile transfers, then xq; wq e-tiles 0-1 (fp16,
        # only v-proj needs them), xv and wo stream last -- pair-0 scores
        # run fp16 straight from the drain so nothing critical queues
        # behind them in the serialized DMA-engine FIFO
        nc.sync.dma_start(out=xk8_sb[...], in_=xk8_r)
        nc.sync.dma_start(out=wq8_sb[...], in_=wq8_r)
        nc.sync.dma_start(out=xq8_sb[...], in_=xq8_r)
        for t in range(0, NE16, 2):
            nc.sync.dma_start(out=xk_sb[:, t:t + 2, :], in_=xk_r[:, t:t + 2, :])
            nc.sync.dma_start(out=wq_sb[:, t + NF8:t + NF8 + 2, :],
                              in_=wq_r[:, t + NF8:t + NF8 + 2, :])
        for t in range(0, NE16, 2):
            nc.sync.dma_start(out=xq_sb[:, t:t + 2, :], in_=xq_r[:, t:t + 2, :])
        nc.sync.dma_start(out=wq_sb[:, 0:NF8, :], in_=wq_r[:, 0:NF8, :])
        for h in range(2):
            sl = slice(h * 4, (h + 1) * 4)
            nc.sync.dma_start(out=xv_sb[:, sl, :], in_=xv_r[:, sl, :])
        nc.sync.dma_start(out=wo_sb[:, :, :], in_=wo_r)

        def body():
            def jmm(n, width=512):
                # junk ramp-keeper matmuls: ~213ns each of always-ready PE
                # work (they only read the zeroed junk tile)
                for _ in range(n):
                    jp = ps_j.tile([P, 512], f32, tag="junk", name="jp")
                    nc.tensor.matmul(jp[:, 0:width], lhsT=junk[:, 0:128],
                                     rhs=junk[:, 0:width],
                                     start=True, stop=True)

            # warm-up: bridge from t=0 until the first k-proj inputs land
            jmm(16)

            for st in range(NST):
                vh = v_sb[st].rearrange("p (h c) -> p h c", c=vw)
                nc.vector.memset(vh[:, :, DH], 1.0)

            # psum slots for projection-phase matmul groups: ps_o always;
            # ps_s only while it isn't busy with scores (head phase)
            rr_state = [0]

            def rr_psum(head=False):
                if head:
                    i = rr_state[0] % 4
                    rr_state[0] += 1
                    if i < 2:
                        t = ps_s.tile([P, S], f32, tag="ps_s", name="ps")
                        return t[:, 0:512]
                return ps_o.tile([P, 512], f32, tag="ps_o", name="ps")

            def proj_qk_group(jt, qc, x16, x8, dst8, bias, head=False):
                # one (jt, qc) projection group: 1 fp8-DR matmul over
                # e-tiles 0-1 + 6 fp16 matmuls over e-tiles 2-7
                ps = rr_psum(head)
                for dp in range(NF8 // 2):
                    nc.tensor.matmul(
                        ps,
                        lhsT=wq8_sb[:, 2 * dp:2 * dp + 2, jt * P:(jt + 1) * P],
                        rhs=x8[:, 2 * dp:2 * dp + 2, qc * 512:(qc + 1) * 512],
                        start=(dp == 0), stop=False,
                        perf_mode=DR,
                    )
                for t in range(NF8, NE):
                    nc.tensor.matmul(
                        ps,
                        lhsT=wq_sb[:, t, jt * P:(jt + 1) * P],
                        rhs=x16[:, t - NF8, qc * 512:(qc + 1) * 512],
                        start=False,
                        stop=(t == NE - 1),
                    )
                d = dst8[:, jt - 1, qc * 512:(qc + 1) * 512]
                if bias is not None:
                    nc.vector.tensor_scalar_add(d, ps, bias[:, jt:jt + 1])
                else:
                    nc.vector.tensor_copy(d, ps)

            def regroup(jt, src8, stage, dstdr):
                # SBUF -> DRAM -> SBUF partition fold 128 -> 32x(2 head,
                # 2 dsub); rides the gpsimd SWDGE queue so neither the SP
                # input stream nor the ACT exp queue serializes against it;
                # src8 holds j-tiles 1..3 at jt-1
                nc.gpsimd.dma_start(out=stage[jt], in_=src8[:, jt - 1, :])
                nc.gpsimd.dma_start(
                    out=dstdr[:, jt - 1, :, :, :],
                    in_=stage[jt].rearrange("(h d q) s -> q h d s", h=2, d=2),
                )

            def proj_v(st):
                ps = rr_psum()
                for t in range(NE):
                    nc.tensor.matmul(
                        ps,
                        lhsT=xv_sb[:, t, st * P:(st + 1) * P],
                        rhs=wq_sb[:, t, :],
                        start=(t == 0),
                        stop=(t == NE - 1),
                    )
                vh = v_sb[st].rearrange("p (h c) -> p h c", c=vw)
                nc.vector.tensor_copy(
                    vh[:, :, 0:DH],
                    ps.rearrange("p (h d) -> p h d", d=DH))

            pt_pairs = [[[], []] for _ in range(NJ)]

            def scores_exp_kt(jt, kt):
                # fp8 DoubleRow scores for the head pair of j-tile jt at
                # key-tile kt: per hh one [128, 1024] psum tile (2 banks)
                # covering both q-chunks; 2 DR matmuls fill it; exp is one
                # wide ACT op
                for hh in range(2):
                    pss = ps_s.tile([P, S], f32, tag="ps_s", name=f"pss{hh}")
                    pt = pt_pool.tile([P, S], f16, tag="pt", name=f"pt{hh}")
                    pt_pairs[jt][hh].append(pt)
                    bp = hh * DH
                    for qc in range(NQ):
                        if jt == 0:
                            nc.tensor.matmul(
                                pss[:, qc * 512:(qc + 1) * 512],
                                lhsT=kt0_sb[bp:bp + DH, kt * P:(kt + 1) * P],
                                rhs=qt0_sb[bp:bp + DH,
                                           qc * 512:(qc + 1) * 512],
                                start=True, stop=True,
                            )
                        else:
                            nc.tensor.matmul(
                                pss[:, qc * 512:(qc + 1) * 512],
                                lhsT=k8dr[:, jt - 1, hh, :,
                                          kt * P:(kt + 1) * P],
                                rhs=q8dr[:, jt - 1, hh, :,
                                         qc * 512:(qc + 1) * 512],
                                start=True, stop=True,
                                perf_mode=DR,
                            )
                    nc.scalar.activation(
                        out=pt, in_=pss, func=AF.Exp, scale=0.125,
                    )

            cq_store = {}

            def av_pair(jt, st):
                # AV for both heads of pair jt at query tile st, packed
                # into one psum bank: head hh occupies cols [hh*65,
                # hh*65+65) (64 dims + denominator from the V ones column)
                pts = pt_pairs[jt]
                po = ps_o.tile([P, 512], f32, tag="ps_o", name="po")
                for kt in range(NE):
                    for hh in range(2):
                        h = 2 * jt + hh
                        nc.tensor.matmul(
                            po[:, hh * vw:(hh + 1) * vw],
                            lhsT=pts[hh][kt][:, st * P:(st + 1) * P],
                            rhs=v_sb[kt][:, h * vw:(h + 1) * vw],
                            start=(kt == 0 and hh == 0),
                            stop=(kt == NE - 1 and hh == 1),
                            skip_group_check=True,
                        )
                cu = cu_pool.tile([P, 2 * vw], f32, tag="cu", name="cu")
                nc.vector.tensor_copy(cu, po[:, 0:2 * vw])
                cq = cq_pool.tile([P, 2, DH], f16, tag="cq", name="cq")
                for hh in range(2):
                    nc.gpsimd.normalize_recip(
                        cq[:, hh, :],
                        cu[:, hh * vw:hh * vw + DH],
                        cu[:, hh * vw + DH:(hh + 1) * vw],
                    )
                cq_store[(jt, st)] = cq

            def transpose_pair(jt, st):
                # [128 q, 64 d] per head -> psum [64 d, 128 q] stacked pair
                cq = cq_store.pop((jt, st))
                pst = ps_t.tile([P, P], f16, tag="ps_t", name="pst")
                for hh in range(2):
                    nc.tensor.matmul(
                        pst[hh * DH:(hh + 1) * DH, :],
                        lhsT=cq[:, hh, :],
                        rhs=ident,
                        is_transpose=True,
                        tile_position=(0, hh * DH),
                        skip_group_check=True,
                    )
                dst = ct_sb[:, jt, st * P:(st + 1) * P]
                if jt == NJ - 1:
                    # tail pair: ACT is free once the exp stream ends
                    nc.scalar.copy(dst, pst)
                else:
                    nc.vector.tensor_copy(dst, pst)

            def av_step(p, st):
                av_pair(p, st)
                if st >= 2:
                    transpose_pair(p, st - 2)
                if st == NST - 1:
                    transpose_pair(p, st - 1)
                    transpose_pair(p, st)

            def op_a(st, ocs=(0, 1)):
                # out-proj pass A: ft0+ft1+ft2 partial -> oa (fp16 staging)
                for oc in ocs:
                    ps = ps_o.tile([P, 512], f32, tag="ps_o", name="ps")
                    for ft in range(3):
                        nc.tensor.matmul(
                            ps,
                            lhsT=ct_sb[:, ft, st * P:(st + 1) * P],
                            rhs=wo_sb[:, ft, oc * 512:(oc + 1) * 512],
                            start=(ft == 0),
                            stop=(ft == 2),
                        )
                    nc.vector.tensor_copy(
                        oa_sb[:, st, oc * 512:(oc + 1) * 512], ps)

            def op_b(st, split_dma=False):
                # out-proj pass B (tail): ft3 + oa -> out, per-oc tiles and
                # stores (drains split across DVE and gpsimd)
                for oc in range(NQ):
                    sl = slice(oc * 512, (oc + 1) * 512)
                    ot = outp.tile([P, 512], f16, tag="ot", name="ot")
                    pst = ps_s.tile([P, S], f32, tag="ps_s", name="ps")
                    ps = pst[:, 0:512]
                    nc.tensor.matmul(
                        ps,
                        lhsT=ct_sb[:, 3, st * P:(st + 1) * P],
                        rhs=wo_sb[:, 3, sl],
                        start=True, stop=False,
                    )
                    nc.tensor.matmul(
                        ps,
                        lhsT=ident,
                        rhs=oa_sb[:, st, sl],
                        start=False, stop=True,
                    )
                    if oc == 0:
                        nc.vector.tensor_copy(ot, ps)
                    else:
                        # ACT is idle in the tail (exp stream done)
                        nc.scalar.copy(ot, ps)
                    nc.sync.dma_start(
                        out=out_d[st * P:(st + 1) * P, sl],
                        in_=ot,
                    )

            # ---- emission ----
            # head: k-proj then q-proj for j-tiles 0,1, tile-major with
            # junk bridges sized to the DMA arrival cadence (per-tile
            # transfer ~0.7us unlocks 4 matmuls ~0.85us; junk fills the
            # start-up slack).  JH tunables were set from the sim trace.
            JH = [6, 0, 0, 0, 0, 0]
            kq01 = [(jt, qc) for jt in (0, 1) for qc in range(NQ)]
            pss_k = {g: rr_psum(True) for g in kq01}
            for dp in range(NF8 // 2):
                for g in kq01:
                    nc.tensor.matmul(
                        pss_k[g],
                        lhsT=wq8_sb[:, 2 * dp:2 * dp + 2, g[0] * P:(g[0] + 1) * P],
                        rhs=xk8_sb[:, 2 * dp:2 * dp + 2, g[1] * 512:(g[1] + 1) * 512],
                        start=(dp == 0), stop=False, perf_mode=DR)
            for t in range(NF8, NE):
                jmm(JH[t - NF8])
                for jt, qc in kq01:
                    nc.tensor.matmul(
                        pss_k[(jt, qc)],
                        lhsT=wq_sb[:, t, jt * P:(jt + 1) * P],
                        rhs=xk_sb[:, t - NF8, qc * 512:(qc + 1) * 512],
                        start=False, stop=(t == NE - 1))
            for jt, qc in kq01:
                if jt == 0:
                    nc.vector.tensor_copy(
                        kt0_sb[:, qc * 512:(qc + 1) * 512], pss_k[(jt, qc)])
                else:
                    nc.vector.tensor_copy(
                        k8_sb[:, jt - 1, qc * 512:(qc + 1) * 512],
                        pss_k[(jt, qc)])
            regroup(1, k8_sb, k_stage, k8dr)
            # q-proj: j-tile 0 alone first so its drain (the exp-stream
            # gate) lands as early as possible after the last xq tile;
            # j-tile 1 follows (its regroup has a whole pair of slack)
            for jts in ((0,), (1,)):
                qg = [(jt, qc) for jt in jts for qc in range(NQ)]
                pss_q = {g: rr_psum(True) for g in qg}
                for dp in range(NF8 // 2):
                    for g in qg:
                        nc.tensor.matmul(
                            pss_q[g],
                            lhsT=wq8_sb[:, 2 * dp:2 * dp + 2,
                                        g[0] * P:(g[0] + 1) * P],
                            rhs=xq8_sb[:, 2 * dp:2 * dp + 2,
                                       g[1] * 512:(g[1] + 1) * 512],
                            start=(dp == 0), stop=False, perf_mode=DR)
                for t in range(NF8, NE):
                    for jt, qc in qg:
                        nc.tensor.matmul(
                            pss_q[(jt, qc)],
                            lhsT=wq_sb[:, t, jt * P:(jt + 1) * P],
                            rhs=xq_sb[:, t - NF8, qc * 512:(qc + 1) * 512],
                            start=False, stop=(t == NE - 1))
                for jt, qc in qg:
                    dst = (qt0_sb[:, qc * 512:(qc + 1) * 512] if jt == 0 else
                           q8_sb[:, jt - 1, qc * 512:(qc + 1) * 512])
                    nc.vector.tensor_scalar_add(dst, pss_q[(jt, qc)],
                                                bq_sb[:, jt:jt + 1])
            regroup(1, q8_sb, q_stage, q8dr)

            # filler thunks woven between scores blocks
            def kq(jt, qc, x16, x8, dst8, bias):
                return lambda: proj_qk_group(jt, qc, x16, x8, dst8, bias)

            def rg(jt, src8, stage, dstdr):
                return lambda: regroup(jt, src8, stage, dstdr)

            def av(p, st):
                return lambda: av_step(p, st)

            def vp(st):
                return lambda: proj_v(st)

            def jk(n):
                return lambda: jmm(n)

            fillers = {
                # pair 0: j-tile-2 projections early (regroups well before
                # pair 2), then the first v-projections
                0: [[kq(2, 0, xk_sb, xk8_sb, k8_sb, None)],
                    [kq(2, 1, xk_sb, xk8_sb, k8_sb, None),
                     rg(2, k8_sb, k_stage, k8dr)],
                    [kq(2, 0, xq_sb, xq8_sb, q8_sb, bq_sb)],
                    [kq(2, 1, xq_sb, xq8_sb, q8_sb, bq_sb),
                     rg(2, q8_sb, q_stage, q8dr)],
                    [vp(0)], [vp(1)], [vp(2)], [vp(3)]],
                # pair 1: rest of v-proj, then AV(0)
                1: [[vp(4)], [vp(5)], [vp(6)], [vp(7)],
                    [av(0, 0), av(0, 1)],
                    [av(0, 2), av(0, 3)],
                    [av(0, 4), av(0, 5)],
                    [av(0, 6), av(0, 7)]],
                # pair 2: j-tile-3 projections early + AV(1)
                2: [[kq(3, 0, xk_sb, xk8_sb, k8_sb, None), av(1, 0)],
                    [kq(3, 1, xk_sb, xk8_sb, k8_sb, None),
                     rg(3, k8_sb, k_stage, k8dr), av(1, 1)],
                    [kq(3, 0, xq_sb, xq8_sb, q8_sb, bq_sb), av(1, 2)],
                    [kq(3, 1, xq_sb, xq8_sb, q8_sb, bq_sb),
                     rg(3, q8_sb, q_stage, q8dr), av(1, 3)],
                    [av(1, 4), jk(4)],
                    [av(1, 5), jk(4)],
                    [av(1, 6), jk(4)],
                    [av(1, 7), jk(4)]],
                # pair 3: AV(2) first (pt ring), then out-proj pass A
                3: [[av(2, 0), av(2, 1), jk(2)],
                    [av(2, 2), av(2, 3), jk(1)],
                    [av(2, 4), av(2, 5), lambda: op_a(0, (0,))],
                    [av(2, 6), av(2, 7), lambda: op_a(0, (1,))],
                    [lambda: op_a(1), lambda: op_a(2, (0,))],
                    [lambda: op_a(2, (1,)), lambda: op_a(3)],
                    [lambda: op_a(4), lambda: op_a(5, (0,))],
                    [lambda: op_a(5, (1,)), lambda: op_a(6)]],
            }
            for p in range(NJ):
                for kt in range(NE):
                    scores_exp_kt(p, kt)
                    for th in fillers[p][kt]:
                        th()

            # tail: AV(3) -> transpose -> out-proj pass B chase (lag 2)
            op_a(7)
            for st in range(NST):
                av_pair(3, st)
                if st >= 1:
                    transpose_pair(3, st - 1)
                if st >= 2:
                    op_b(st - 2)
            transpose_pair(3, NST - 1)
            for st in range(NST - 2, NST):
                op_b(st)

        for _ in range(reps):
            body()

    nc.finalize()
    return nc


def _get_nc(reps=1):
    key = ("nc", reps)
    if key not in _CACHE:
        _CACHE[key] = _build_program(reps)
    return _CACHE[key]


def make_in_maps(queries, keys, values, Wq_w, Wq_b, Wo_w, Wo_b):
    in_maps = []
    cut = NF8 * P
    for c in range(NCORES):
        b, g = c // 2, c % 2
        js = slice(g * EH, (g + 1) * EH)
        qT = np.ascontiguousarray(queries[b].T)
        kT = np.ascontiguousarray(keys[b].T)
        wT = np.ascontiguousarray(Wq_w[js, :].T)
        in_maps.append({
            "xq_t": qT[cut:].astype(F16),
            "xk_t": kT[cut:].astype(F16),
            "xv_t": np.ascontiguousarray(values[b].T).astype(F16),
            "xq8_t": qT[:cut].astype(F8),
            "xk8_t": kT[:cut].astype(F8),
            "wq_t": wT.astype(F16),
            "wq8_t": wT[:cut].astype(F8),
            "wo_t": np.ascontiguousarray(Wo_w[:, js].T).astype(F16),
            "bq": np.ascontiguousarray(Wq_b[js].reshape(NJ, P).T),
        })
    return in_maps


def assemble_output(results, Wq_b, Wo_w, Wo_b):
    bias_total = (Wo_w @ Wq_b + Wo_b).astype(np.float32)
    out = np.empty((B, S, E), np.float32)
    for b in range(B):
        out[b] = (results[2 * b]["out_partial"].astype(np.float32)
                  + results[2 * b + 1]["out_partial"].astype(np.float32))
    out += bias_total
    return out


def kernel(queries, keys, values, Wq_w, Wq_b, Wo_w, Wo_b, num_heads):
    from concourse.bass_utils import run_bass_kernel_spmd

    queries = np.asarray(queries, np.float32)
    keys = np.asarray(keys, np.float32)
    values = np.asarray(values, np.float32)
    Wq_w = np.asarray(Wq_w, np.float32)
    Wq_b = np.asarray(Wq_b, np.float32)
    Wo_w = np.asarray(Wo_w, np.float32)
    Wo_b = np.asarray(Wo_b, np.float32)
    assert int(num_heads) == H

    nc = _get_nc()
    in_maps = make_in_maps(queries, keys, values, Wq_w, Wq_b, Wo_w, Wo_b)
    res = run_bass_kernel_spmd(nc, in_maps, core_ids=list(range(NCORES)))
    _CACHE["last_results"] = res
    return assemble_output(res.results, Wq_b, Wo_w, Wo_b)


# revision 5
# speedup vs baseline: 1.0607x; 1.0054x over previous
"""Trainium2 Bass kernel for nn_MultiHeadAttention_79534204387726.

Reference computation (B=4, S=1024, E=1024, H=16, dh=64):
    q/k/v = proj(x) = x @ Wq_w.T + Wq_b       (same Wq applied to q, k, v)
    scores = q @ k.T / 8 per head; attn = softmax(scores)
    out = (attn @ v).concat_heads @ Wo_w.T + Wo_b

Sharding (8 cores): core c -> batch b = c//2, head-group g = c%2 (8 heads,
512 features). Host sums the two partial outputs per batch and adds the
folded bias (Wo_b + Wo@Wq_b; K-bias is softmax-invariant and dropped).

v2 design (cost-model driven, TimelineSim):
  - The kernel is paced by the ACT exp stream (64 x [128,1024] exp tiles =
    66.4us, the single-engine floor).  Emission weaves ~1.65us of filler PE
    work between each scores(jt, kt) block so the PE (89us of work) stays
    busy under the exp cadence: v-proj and j-tile-2/3 projections fill the
    early pairs, AV + transposes of the previous pair and early out-proj
    passes fill the later ones.
  - scores matmuls run in fp8e4m3 DoubleRow (0.5 cycles/row): q/k are
    quantized to fp8 during the psum drain (q with bias add), staged to
    DRAM, and reloaded as [32, head, dsub, S] (the 128->32 partition fold
    needs a DMA round trip).  Those DMAs ride the ACT DGE queue so they
    don't queue behind the input loads on SP.
  - The first half of the q/k projection contraction (e-tiles 0-3) also
    runs in fp8 DoubleRow from host-quantized x8/wq8.  Measured end-to-end
    rel_err 0.0169 (gate 2e-2; pair 0 stays fp16 which also skips the
    regroup round-trip on the exp-stream critical path).
  - AV is flipped to out[q, d]: lhsT = exp(scores^T) [k, q] chunk, rhs = V
    [k, 65] per head (64 dims + ones column -> denominator lands in psum
    col 64 per q row): 65 free-cols/instruction instead of 512.  Two heads
    pack into one psum bank (second head's first write exploits the lazy
    2KB zero-region).
  - softmax normalize runs on gpsimd (normalize_recip: divide + fp16 cast).
  - C comes out as [q, d]; a PE transpose (identity matmul) restores the
    [feature, seq] layout the out-projection needs as lhsT.
  - out-proj is split: ft0+ft1 partials during pair 2, += ft2 during pair
    3 (fp16 SBUF staging, identity-matmul re-accumulation), ft3 + final
    add in the tail so only ~1.4us/st of PE work remains after the last
    exp.
"""

import numpy as np
import ml_dtypes

B, S, E, H = 4, 1024, 1024, 16
NCORES = 8
EH = E // 2        # 512 features per head-group
NHG = H // 2       # 8 heads per group
DH = E // H        # 64
P = 128
NE = E // P        # 8 e-tiles over full E
NF8 = 4            # e-tiles 0-3 of the q/k projection contract in fp8 DR
NE16 = NE - NF8    # e-tiles 2-7 in fp16
NJ = EH // P       # 4 j-tiles over the group's 512 features
NQ = S // 512      # 2 query/sequence chunks of 512
NST = S // P       # 8 sequence tiles of 128
F16 = np.float16
F8 = ml_dtypes.float8_e4m3fn

_CACHE = {}


def _build_program(reps=1):
    import concourse.tile as tile
    from concourse import bacc, mybir
    from concourse.masks import make_identity
    from contextlib import ExitStack

    f32 = mybir.dt.float32
    f16 = mybir.dt.float16
    f8 = mybir.dt.float8e4
    AF = mybir.ActivationFunctionType
    DR = mybir.MatmulPerfMode.DoubleRow

    nc = bacc.Bacc(
        "TRN2",
        target_bir_lowering=False,
        debug=False,
        num_devices=NCORES,
    )

    # fp16 x for q/k carries only e-tiles 2-7; tiles 0-1 come as fp8
    xq_t = nc.dram_tensor("xq_t", [NE16 * P, S], f16, kind="ExternalInput")
    xk_t = nc.dram_tensor("xk_t", [NE16 * P, S], f16, kind="ExternalInput")
    xv_t = nc.dram_tensor("xv_t", [E, S], f16, kind="ExternalInput")
    xq8_t = nc.dram_tensor("xq8_t", [NF8 * P, S], f8, kind="ExternalInput")
    xk8_t = nc.dram_tensor("xk8_t", [NF8 * P, S], f8, kind="ExternalInput")
    wq_t = nc.dram_tensor("wq_t", [E, EH], f16, kind="ExternalInput")
    wq8_t = nc.dram_tensor("wq8_t", [NF8 * P, EH], f8, kind="ExternalInput")
    wo_t = nc.dram_tensor("wo_t", [EH, E], f16, kind="ExternalInput")
    bq = nc.dram_tensor("bq", [P, NJ], f32, kind="ExternalInput")
    out_d = nc.dram_tensor("out_partial", [S, E], f16, kind="ExternalOutput")
    # DRAM staging for the fp8 partition regroup (128 -> 32x4)
    q_stage = nc.dram_tensor("q_stage", [NJ, P, S], f8, kind="Internal")
    k_stage = nc.dram_tensor("k_stage", [NJ, P, S], f8, kind="Internal")

    vw = DH + 1        # per-head V columns incl. the ones column

    with tile.TileContext(nc) as tc, ExitStack() as ctx:
        const = ctx.enter_context(tc.tile_pool(name="const", bufs=1))
        pt_pool = ctx.enter_context(tc.tile_pool(name="pt", bufs=32))
        cu_pool = ctx.enter_context(tc.tile_pool(name="cu", bufs=6))
        cq_pool = ctx.enter_context(tc.tile_pool(name="cq", bufs=6))
        outp = ctx.enter_context(tc.tile_pool(name="outp", bufs=6))
        ps_s = ctx.enter_context(tc.tile_pool(name="ps_s", bufs=2, space="PSUM"))
        ps_o = ctx.enter_context(tc.tile_pool(name="ps_o", bufs=2, space="PSUM"))
        ps_t = ctx.enter_context(tc.tile_pool(name="ps_t", bufs=1, space="PSUM"))
        # junk ramp-keeper matmuls write here: they keep the PE engine busy
        # through known feed gaps so the p-state never drops (matmul cost
        # is priced at dispatch with pe_busy_start reset on any engine
        # idle); single buf -> junk serializes on itself only
        ps_j = ctx.enter_context(tc.tile_pool(name="ps_j", bufs=1, space="PSUM"))

        # ---- resident SBUF tensors ----
        wq_sb = const.tile([P, NE, EH], f16)     # full W (v-proj + qk 2-7)
        wq8_sb = const.tile([P, NF8, EH], f8)
        wo_sb = const.tile([P, NJ, E], f16)      # [p, f-tile, o]
        bq_sb = const.tile([P, NJ], f32)
        junk = const.tile([P, 512], f16)
        ident = const.tile([P, P], f16)
        xq_sb = const.tile([P, NE16, S], f16)
        xk_sb = const.tile([P, NE16, S], f16)
        xq8_sb = const.tile([P, NF8, S], f8)
        xk8_sb = const.tile([P, NF8, S], f8)
        xv_sb = const.tile([P, NE, S], f16)
        # fp8 q/k staging (proj drain output, pre-regroup) [p=j, jt, s]
        q8_sb = const.tile([P, NJ - 1, S], f8)
        k8_sb = const.tile([P, NJ - 1, S], f8)
        # pair 0 (j-tile 0) keeps q/k in fp16 and runs fp16 scores straight
        # from the drain: no DRAM regroup round-trip on the critical path
        qt0_sb = const.tile([P, S], f16)
        kt0_sb = const.tile([P, S], f16)
        # DR-layout q/k [32, jt-1, head, dsub, s] (j-tiles 1..3 only;
        # pair 0 runs fp16)
        q8dr = const.tile([32, NJ - 1, 2, 2, S], f8)
        k8dr = const.tile([32, NJ - 1, 2, 2, S], f8)
        # V tiles [key-tile][p=k, 8 heads x (dh + ones col)]
        v_sb = [const.tile([P, NHG * vw], f16, tag=f"v{st}", name=f"v{st}")
                for st in range(NST)]
        # transposed attention output C^T [p=f, f-tile, s] fp16
        ct_sb = const.tile([P, NJ, S], f16)
        # out-proj partial accumulator [p=s, st, o] fp16
        oa_sb = const.tile([P, NST, S], f16)

        nc.sync.dma_start(out=bq_sb[:, :], in_=bq[:, :])
        nc.vector.memset(junk, 0.0)
        make_identity(nc, ident)
        wq_r = wq_t.rearrange("(t p) j -> p t j", p=P)
        wq8_r = wq8_t.rearrange("(t p) j -> p t j", p=P)
        xk_r = xk_t.rearrange("(t p) s -> p t s", p=P)
        xq_r = xq_t.rearrange("(t p) s -> p t s", p=P)
        xk8_r = xk8_t.rearrange("(t p) s -> p t s", p=P)
        xq8_r = xq8_t.rearrange("(t p) s -> p t s", p=P)
        xv_r = xv_t.rearrange("(t p) s -> p t s", p=P)
        wo_r = wo_t.rearrange("(t p) o -> p t o", p=P)
        # input DMA order tuned so the k/q projections can stream in
        # two-tile waves: fp8 parts first (tiny), then alternating
        # (xk, wq) two-tile transfers, then xq; wq e-tiles 0-1 (fp16,
        # only v-proj needs them), xv and wo stream last -- pair-0 scores
        # run fp16 straight from the drain so nothing critical queues
        # behind them in the serialized DMA-engine FIFO
        nc.sync.dma_start(out=xq8_sb[...], in_=xq8_r)
        nc.sync.dma_start(out=wq8_sb[...], in_=wq8_r)
        nc.sync.dma_start(out=xk8_sb[...], in_=xk8_r)
        for t in range(0, NE16, 2):
            nc.sync.dma_start(out=xq_sb[:, t:t + 2, :], in_=xq_r[:, t:t + 2, :])
            nc.sync.dma_start(out=wq_sb[:, t + NF8:t + NF8 + 2, :],
                              in_=wq_r[:, t + NF8:t + NF8 + 2, :])
        for t in range(0, NE16, 2):
            nc.sync.dma_start(out=xk_sb[:, t:t + 2, :], in_=xk_r[:, t:t + 2, :])
        nc.sync.dma_start(out=wq_sb[:, 0:NF8, :], in_=wq_r[:, 0:NF8, :])
        for h in range(2):
            sl = slice(h * 4, (h + 1) * 4)
            nc.sync.dma_start(out=xv_sb[:, sl, :], in_=xv_r[:, sl, :])
        nc.sync.dma_start(out=wo_sb[:, :, :], in_=wo_r)

        def body():
            def jmm(n, width=512):
                # junk ramp-keeper matmuls: ~213ns each of always-ready PE
                # work (they only read the zeroed junk tile)
                for _ in range(n):
                    jp = ps_j.tile([P, 512], f32, tag="junk", name="jp")
                    nc.tensor.matmul(jp[:, 0:width], lhsT=junk[:, 0:128],
                                     rhs=junk[:, 0:width],
                                     start=True, stop=True)

            # warm-up: bridge from t=0 until the first k-proj inputs land
            jmm(16)

            for st in range(NST):
                vh = v_sb[st].rearrange("p (h c) -> p h c", c=vw)
                nc.vector.memset(vh[:, :, DH], 1.0)

            # psum slots for projection-phase matmul groups: ps_o always;
            # ps_s only while it isn't busy with scores (head phase)
            rr_state = [0]

            def rr_psum(head=False):
                if head:
                    i = rr_state[0] % 4
                    rr_state[0] += 1
                    if i < 2:
                        t = ps_s.tile([P, S], f32, tag="ps_s", name="ps")
                        return t[:, 0:512]
                return ps_o.tile([P, 512], f32, tag="ps_o", name="ps")

            def proj_qk_group(jt, qc, x16, x8, dst8, bias, head=False):
                # one (jt, qc) projection group: 1 fp8-DR matmul over
                # e-tiles 0-1 + 6 fp16 matmuls over e-tiles 2-7
                ps = rr_psum(head)
                for dp in range(NF8 // 2):
                    nc.tensor.matmul(
                        ps,
                        lhsT=wq8_sb[:, 2 * dp:2 * dp + 2, jt * P:(jt + 1) * P],
                        rhs=x8[:, 2 * dp:2 * dp + 2, qc * 512:(qc + 1) * 512],
                        start=(dp == 0), stop=False,
                        perf_mode=DR,
                    )
                for t in range(NF8, NE):
                    nc.tensor.matmul(
                        ps,
                        lhsT=wq_sb[:, t, jt * P:(jt + 1) * P],
                        rhs=x16[:, t - NF8, qc * 512:(qc + 1) * 512],
                        start=False,
                        stop=(t == NE - 1),
                    )
                d = dst8[:, jt - 1, qc * 512:(qc + 1) * 512]
                if bias is not None:
                    nc.vector.tensor_scalar_add(d, ps, bias[:, jt:jt + 1])
                else:
                    nc.vector.tensor_copy(d, ps)

            def regroup(jt, src8, stage, dstdr):
                # SBUF -> DRAM -> SBUF partition fold 128 -> 32x(2 head,
                # 2 dsub); rides the gpsimd SWDGE queue so neither the SP
                # input stream nor the ACT exp queue serializes against it;
                # src8 holds j-tiles 1..3 at jt-1
                nc.gpsimd.dma_start(out=stage[jt], in_=src8[:, jt - 1, :])
                nc.gpsimd.dma_start(
                    out=dstdr[:, jt - 1, :, :, :],
                    in_=stage[jt].rearrange("(h d q) s -> q h d s", h=2, d=2),
                )

            def proj_v(st):
                ps = rr_psum()
                for t in range(NE):
                    nc.tensor.matmul(
                        ps,
                        lhsT=xv_sb[:, t, st * P:(st + 1) * P],
                        rhs=wq_sb[:, t, :],
                        start=(t == 0),
                        stop=(t == NE - 1),
                    )
                vh = v_sb[st].rearrange("p (h c) -> p h c", c=vw)
                nc.vector.tensor_copy(
                    vh[:, :, 0:DH],
                    ps.rearrange("p (h d) -> p h d", d=DH))

            pt_pairs = [[[], []] for _ in range(NJ)]

            def scores_exp_kt(jt, kt):
                # fp8 DoubleRow scores for the head pair of j-tile jt at
                # key-tile kt: per hh one [128, 1024] psum tile (2 banks)
                # covering both q-chunks; 2 DR matmuls fill it; exp is one
                # wide ACT op
                for hh in range(2):
                    pss = ps_s.tile([P, S], f32, tag="ps_s", name=f"pss{hh}")
                    pt = pt_pool.tile([P, S], f16, tag="pt", name=f"pt{hh}")
                    pt_pairs[jt][hh].append(pt)
                    bp = hh * DH
                    for qc in range(NQ):
                        if jt == 0:
                            nc.tensor.matmul(
                                pss[:, qc * 512:(qc + 1) * 512],
                                lhsT=kt0_sb[bp:bp + DH, kt * P:(kt + 1) * P],
                                rhs=qt0_sb[bp:bp + DH,
                                           qc * 512:(qc + 1) * 512],
                                start=True, stop=True,
                            )
                        else:
                            nc.tensor.matmul(
                                pss[:, qc * 512:(qc + 1) * 512],
                                lhsT=k8dr[:, jt - 1, hh, :,
                                          kt * P:(kt + 1) * P],
                                rhs=q8dr[:, jt - 1, hh, :,
                                         qc * 512:(qc + 1) * 512],
                                start=True, stop=True,
                                perf_mode=DR,
                            )
                    nc.scalar.activation(
                        out=pt, in_=pss, func=AF.Exp, scale=0.125,
                    )

            cq_store = {}

            def av_pair(jt, st):
                # AV for both heads of pair jt at query tile st, packed
                # into one psum bank: head hh occupies cols [hh*65,
                # hh*65+65) (64 dims + denominator from the V ones column)
                pts = pt_pairs[jt]
                po = ps_o.tile([P, 512], f32, tag="ps_o", name="po")
                for kt in range(NE):
                    for hh in range(2):
                        h = 2 * jt + hh
                        nc.tensor.matmul(
                            po[:, hh * vw:(hh + 1) * vw],
                            lhsT=pts[hh][kt][:, st * P:(st + 1) * P],
                            rhs=v_sb[kt][:, h * vw:(h + 1) * vw],
                            start=(kt == 0 and hh == 0),
                            stop=(kt == NE - 1 and hh == 1),
                            skip_group_check=True,
                        )
                cu = cu_pool.tile([P, 2 * vw], f32, tag="cu", name="cu")
                nc.vector.tensor_copy(cu, po[:, 0:2 * vw])
                cq = cq_pool.tile([P, 2, DH], f16, tag="cq", name="cq")
                for hh in range(2):
                    nc.gpsimd.normalize_recip(
                        cq[:, hh, :],
                        cu[:, hh * vw:hh * vw + DH],
                        cu[:, hh * vw + DH:(hh + 1) * vw],
                    )
                cq_store[(jt, st)] = cq

            def transpose_pair(jt, st):
                # [128 q, 64 d] per head -> psum [64 d, 128 q] stacked pair
                cq = cq_store.pop((jt, st))
                pst = ps_t.tile([P, P], f16, tag="ps_t", name="pst")
                for hh in range(2):
                    nc.tensor.matmul(
                        pst[hh * DH:(hh + 1) * DH, :],
                        lhsT=cq[:, hh, :],
                        rhs=ident,
                        is_transpose=True,
                        tile_position=(0, hh * DH),
                        skip_group_check=True,
                    )
                dst = ct_sb[:, jt, st * P:(st + 1) * P]
                if jt == NJ - 1:
                    # tail pair: ACT is free once the exp stream ends
                    nc.scalar.copy(dst, pst)
                else:
                    nc.vector.tensor_copy(dst, pst)

            def av_step(p, st):
                av_pair(p, st)
                if st >= 2:
                    transpose_pair(p, st - 2)
                if st == NST - 1:
                    transpose_pair(p, st - 1)
                    transpose_pair(p, st)

            def op_a(st, ocs=(0, 1)):
                # out-proj pass A: ft0+ft1+ft2 partial -> oa (fp16 staging)
                for oc in ocs:
                    ps = ps_o.tile([P, 512], f32, tag="ps_o", name="ps")
                    for ft in range(3):
                        nc.tensor.matmul(
                            ps,
                            lhsT=ct_sb[:, ft, st * P:(st + 1) * P],
                            rhs=wo_sb[:, ft, oc * 512:(oc + 1) * 512],
                            start=(ft == 0),
                            stop=(ft == 2),
                        )
                    nc.vector.tensor_copy(
                        oa_sb[:, st, oc * 512:(oc + 1) * 512], ps)

            def op_b(st, split_dma=False):
                # out-proj pass B (tail): ft3 + oa -> out, per-oc tiles and
                # stores (drains split across DVE and gpsimd)
                for oc in range(NQ):
                    sl = slice(oc * 512, (oc + 1) * 512)
                    ot = outp.tile([P, 512], f16, tag="ot", name="ot")
                    pst = ps_s.tile([P, S], f32, tag="ps_s", name="ps")
                    ps = pst[:, 0:512]
                    nc.tensor.matmul(
                        ps,
                        lhsT=ct_sb[:, 3, st * P:(st + 1) * P],
                        rhs=wo_sb[:, 3, sl],
                        start=True, stop=False,
                    )
                    nc.tensor.matmul(
                        ps,
                        lhsT=ident,
                        rhs=oa_sb[:, st, sl],
                        start=False, stop=True,
                    )
                    if oc == 0:
                        nc.vector.tensor_copy(ot, ps)
                    else:
                        # ACT is idle in the tail (exp stream done)
                        nc.scalar.copy(ot, ps)
                    nc.sync.dma_start(
                        out=out_d[st * P:(st + 1) * P, sl],
                        in_=ot,
                    )

            # ---- emission ----
            # head: k-proj then q-proj for j-tiles 0,1, tile-major with
            # junk bridges sized to the DMA arrival cadence (per-tile
            # transfer ~0.7us unlocks 4 matmuls ~0.85us; junk fills the
            # start-up slack).  JH tunables were set from the sim trace.
            JH = [6, 0, 0, 0, 0, 0]
            # q-proj first (xq now leads the load stream): j-tile 0 alone so
            # its fp16 drain lands earliest, then j-tile 1
            for jts in ((0,), (1,)):
                qg = [(jt, qc) for jt in jts for qc in range(NQ)]
                pss_q = {g: rr_psum(True) for g in qg}
                if jts == (0,):
                    jmm(JH[0])
                for dp in range(NF8 // 2):
                    for g in qg:
                        nc.tensor.matmul(
                            pss_q[g],
                            lhsT=wq8_sb[:, 2 * dp:2 * dp + 2,
                                        g[0] * P:(g[0] + 1) * P],
                            rhs=xq8_sb[:, 2 * dp:2 * dp + 2,
                                       g[1] * 512:(g[1] + 1) * 512],
                            start=(dp == 0), stop=False, perf_mode=DR)
                for t in range(NF8, NE):
                    for jt, qc in qg:
                        nc.tensor.matmul(
                            pss_q[(jt, qc)],
                            lhsT=wq_sb[:, t, jt * P:(jt + 1) * P],
                            rhs=xq_sb[:, t - NF8, qc * 512:(qc + 1) * 512],
                            start=False, stop=(t == NE - 1))
                for jt, qc in qg:
                    dst = (qt0_sb[:, qc * 512:(qc + 1) * 512] if jt == 0 else
                           q8_sb[:, jt - 1, qc * 512:(qc + 1) * 512])
                    nc.vector.tensor_scalar_add(dst, pss_q[(jt, qc)],
                                                bq_sb[:, jt:jt + 1])
            regroup(1, q8_sb, q_stage, q8dr)
            # k-proj: j-tile 0 wave + drain first (gates the exp stream)
            for jts in ((0,), (1,)):
                kg = [(jt, qc) for jt in jts for qc in range(NQ)]
                pss_k = {g: rr_psum(True) for g in kg}
                for dp in range(NF8 // 2):
                    for g in kg:
                        nc.tensor.matmul(
                            pss_k[g],
                            lhsT=wq8_sb[:, 2 * dp:2 * dp + 2,
                                        g[0] * P:(g[0] + 1) * P],
                            rhs=xk8_sb[:, 2 * dp:2 * dp + 2,
                                       g[1] * 512:(g[1] + 1) * 512],
                            start=(dp == 0), stop=False, perf_mode=DR)
                for t in range(NF8, NE):
                    for jt, qc in kg:
                        nc.tensor.matmul(
                            pss_k[(jt, qc)],
                            lhsT=wq_sb[:, t, jt * P:(jt + 1) * P],
                            rhs=xk_sb[:, t - NF8, qc * 512:(qc + 1) * 512],
                            start=False, stop=(t == NE - 1))
                for jt, qc in kg:
                    if jt == 0:
                        nc.vector.tensor_copy(
                            kt0_sb[:, qc * 512:(qc + 1) * 512],
                            pss_k[(jt, qc)])
                    else:
                        nc.vector.tensor_copy(
                            k8_sb[:, jt - 1, qc * 512:(qc + 1) * 512],
                            pss_k[(jt, qc)])
            regroup(1, k8_sb, k_stage, k8dr)

            # filler thunks woven between scores blocks
            def kq(jt, qc, x16, x8, dst8, bias):
                return lambda: proj_qk_group(jt, qc, x16, x8, dst8, bias)

            def rg(jt, src8, stage, dstdr):
                return lambda: regroup(jt, src8, stage, dstdr)

            def av(p, st):
                return lambda: av_step(p, st)

            def vp(st):
                return lambda: proj_v(st)

            def jk(n):
                return lambda: jmm(n)

            fillers = {
                # pair 0: j-tile-2 projections early (regroups well before
                # pair 2), then the first v-projections
                0: [[kq(2, 0, xk_sb, xk8_sb, k8_sb, None)],
                    [kq(2, 1, xk_sb, xk8_sb, k8_sb, None),
                     rg(2, k8_sb, k_stage, k8dr)],
                    [kq(2, 0, xq_sb, xq8_sb, q8_sb, bq_sb)],
                    [kq(2, 1, xq_sb, xq8_sb, q8_sb, bq_sb),
                     rg(2, q8_sb, q_stage, q8dr)],
                    [vp(0)], [vp(1)], [vp(2)], [vp(3)]],
                # pair 1: rest of v-proj, then AV(0)
                1: [[vp(4)], [vp(5)], [vp(6)], [vp(7)],
                    [av(0, 0), av(0, 1)],
                    [av(0, 2), av(0, 3)],
                    [av(0, 4), av(0, 5)],
                    [av(0, 6), av(0, 7)]],
                # pair 2: j-tile-3 projections early + AV(1)
                2: [[kq(3, 0, xk_sb, xk8_sb, k8_sb, None), av(1, 0)],
                    [kq(3, 1, xk_sb, xk8_sb, k8_sb, None),
                     rg(3, k8_sb, k_stage, k8dr), av(1, 1)],
                    [kq(3, 0, xq_sb, xq8_sb, q8_sb, bq_sb), av(1, 2)],
                    [kq(3, 1, xq_sb, xq8_sb, q8_sb, bq_sb),
                     rg(3, q8_sb, q_stage, q8dr), av(1, 3)],
                    [av(1, 4), jk(4)],
                    [av(1, 5), jk(4)],
                    [av(1, 6), jk(4)],
                    [av(1, 7), jk(4)]],
                # pair 3: AV(2) first (pt ring), then out-proj pass A
                3: [[av(2, 0), av(2, 1), jk(2)],
                    [av(2, 2), av(2, 3), jk(1)],
                    [av(2, 4), av(2, 5), lambda: op_a(0, (0,))],
                    [av(2, 6), av(2, 7), lambda: op_a(0, (1,))],
                    [lambda: op_a(1), lambda: op_a(2, (0,))],
                    [lambda: op_a(2, (1,)), lambda: op_a(3)],
                    [lambda: op_a(4), lambda: op_a(5, (0,))],
                    [lambda: op_a(5, (1,)), lambda: op_a(6)]],
            }
            for p in range(NJ):
                for kt in range(NE):
                    scores_exp_kt(p, kt)
                    for th in fillers[p][kt]:
                        th()

            # tail: AV(3) -> transpose -> out-proj pass B chase (lag 2)
            op_a(7)
            for st in range(NST):
                av_pair(3, st)
                if st >= 1:
                    transpose_pair(3, st - 1)
                if st >= 2:
                    op_b(st - 2)
            transpose_pair(3, NST - 1)
            for st in range(NST - 2, NST):
                op_b(st)

        for _ in range(reps):
            body()

    nc.finalize()
    return nc


def _get_nc(reps=1):
    key = ("nc", reps)
    if key not in _CACHE:
        _CACHE[key] = _build_program(reps)
    return _CACHE[key]


def make_in_maps(queries, keys, values, Wq_w, Wq_b, Wo_w, Wo_b):
    in_maps = []
    cut = NF8 * P
    for c in range(NCORES):
        b, g = c // 2, c % 2
        js = slice(g * EH, (g + 1) * EH)
        qT = np.ascontiguousarray(queries[b].T)
        kT = np.ascontiguousarray(keys[b].T)
        wT = np.ascontiguousarray(Wq_w[js, :].T)
        in_maps.append({
            "xq_t": qT[cut:].astype(F16),
            "xk_t": kT[cut:].astype(F16),
            "xv_t": np.ascontiguousarray(values[b].T).astype(F16),
            "xq8_t": qT[:cut].astype(F8),
            "xk8_t": kT[:cut].astype(F8),
            "wq_t": wT.astype(F16),
            "wq8_t": wT[:cut].astype(F8),
            "wo_t": np.ascontiguousarray(Wo_w[:, js].T).astype(F16),
            "bq": np.ascontiguousarray(Wq_b[js].reshape(NJ, P).T),
        })
    return in_maps


def assemble_output(results, Wq_b, Wo_w, Wo_b):
    bias_total = (Wo_w @ Wq_b + Wo_b).astype(np.float32)
    out = np.empty((B, S, E), np.float32)
    for b in range(B):
        out[b] = (results[2 * b]["out_partial"].astype(np.float32)
                  + results[2 * b + 1]["out_partial"].astype(np.float32))
    out += bias_total
    return out


def kernel(queries, keys, values, Wq_w, Wq_b, Wo_w, Wo_b, num_heads):
    from concourse.bass_utils import run_bass_kernel_spmd

    queries = np.asarray(queries, np.float32)
    keys = np.asarray(keys, np.float32)
    values = np.asarray(values, np.float32)
    Wq_w = np.asarray(Wq_w, np.float32)
    Wq_b = np.asarray(Wq_b, np.float32)
    Wo_w = np.asarray(Wo_w, np.float32)
    Wo_b = np.asarray(Wo_b, np.float32)
    assert int(num_heads) == H

    nc = _get_nc()
    in_maps = make_in_maps(queries, keys, values, Wq_w, Wq_b, Wo_w, Wo_b)
    res = run_bass_kernel_spmd(nc, in_maps, core_ids=list(range(NCORES)))
    _CACHE["last_results"] = res
    return assemble_output(res.results, Wq_b, Wo_w, Wo_b)


# revision 6
# speedup vs baseline: 1.0617x; 1.0009x over previous
"""Trainium2 Bass kernel for nn_MultiHeadAttention_79534204387726.

Reference computation (B=4, S=1024, E=1024, H=16, dh=64):
    q/k/v = proj(x) = x @ Wq_w.T + Wq_b       (same Wq applied to q, k, v)
    scores = q @ k.T / 8 per head; attn = softmax(scores)
    out = (attn @ v).concat_heads @ Wo_w.T + Wo_b

Sharding (8 cores): core c -> batch b = c//2, head-group g = c%2 (8 heads,
512 features). Host sums the two partial outputs per batch and adds the
folded bias (Wo_b + Wo@Wq_b; K-bias is softmax-invariant and dropped).

v2 design (cost-model driven, TimelineSim):
  - The kernel is paced by the ACT exp stream (64 x [128,1024] exp tiles =
    66.4us, the single-engine floor).  Emission weaves ~1.65us of filler PE
    work between each scores(jt, kt) block so the PE (89us of work) stays
    busy under the exp cadence: v-proj and j-tile-2/3 projections fill the
    early pairs, AV + transposes of the previous pair and early out-proj
    passes fill the later ones.
  - scores matmuls run in fp8e4m3 DoubleRow (0.5 cycles/row): q/k are
    quantized to fp8 during the psum drain (q with bias add), staged to
    DRAM, and reloaded as [32, head, dsub, S] (the 128->32 partition fold
    needs a DMA round trip).  Those DMAs ride the ACT DGE queue so they
    don't queue behind the input loads on SP.
  - The first half of the q/k projection contraction (e-tiles 0-3) also
    runs in fp8 DoubleRow from host-quantized x8/wq8.  Measured end-to-end
    rel_err 0.0169 (gate 2e-2; pair 0 stays fp16 which also skips the
    regroup round-trip on the exp-stream critical path).
  - AV is flipped to out[q, d]: lhsT = exp(scores^T) [k, q] chunk, rhs = V
    [k, 65] per head (64 dims + ones column -> denominator lands in psum
    col 64 per q row): 65 free-cols/instruction instead of 512.  Two heads
    pack into one psum bank (second head's first write exploits the lazy
    2KB zero-region).
  - softmax normalize runs on gpsimd (normalize_recip: divide + fp16 cast).
  - C comes out as [q, d]; a PE transpose (identity matmul) restores the
    [feature, seq] layout the out-projection needs as lhsT.
  - out-proj is split: ft0+ft1 partials during pair 2, += ft2 during pair
    3 (fp16 SBUF staging, identity-matmul re-accumulation), ft3 + final
    add in the tail so only ~1.4us/st of PE work remains after the last
    exp.
"""

import numpy as np
import ml_dtypes

B, S, E, H = 4, 1024, 1024, 16
NCORES = 8
EH = E // 2        # 512 features per head-group
NHG = H // 2       # 8 heads per group
DH = E // H        # 64
P = 128
NE = E // P        # 8 e-tiles over full E
NF8 = 4            # e-tiles 0-3 of the q/k projection contract in fp8 DR
NE16 = NE - NF8    # e-tiles 2-7 in fp16
NJ = EH // P       # 4 j-tiles over the group's 512 features
NQ = S // 512      # 2 query/sequence chunks of 512
NST = S // P       # 8 sequence tiles of 128
F16 = np.float16
F8 = ml_dtypes.float8_e4m3fn

_CACHE = {}


def _build_program(reps=1):
    import concourse.tile as tile
    from concourse import bacc, mybir
    from concourse.masks import make_identity
    from contextlib import ExitStack

    f32 = mybir.dt.float32
    f16 = mybir.dt.float16
    f8 = mybir.dt.float8e4
    AF = mybir.ActivationFunctionType
    DR = mybir.MatmulPerfMode.DoubleRow

    nc = bacc.Bacc(
        "TRN2",
        target_bir_lowering=False,
        debug=False,
        num_devices=NCORES,
    )

    # fp16 x for q/k carries only e-tiles 2-7; tiles 0-1 come as fp8
    xq_t = nc.dram_tensor("xq_t", [NE16 * P, S], f16, kind="ExternalInput")
    xk_t = nc.dram_tensor("xk_t", [NE16 * P, S], f16, kind="ExternalInput")
    xv_t = nc.dram_tensor("xv_t", [E, S], f16, kind="ExternalInput")
    xq8_t = nc.dram_tensor("xq8_t", [NF8 * P, S], f8, kind="ExternalInput")
    xk8_t = nc.dram_tensor("xk8_t", [NF8 * P, S], f8, kind="ExternalInput")
    wq_t = nc.dram_tensor("wq_t", [E, EH], f16, kind="ExternalInput")
    wq8_t = nc.dram_tensor("wq8_t", [NF8 * P, EH], f8, kind="ExternalInput")
    wo_t = nc.dram_tensor("wo_t", [EH, E], f16, kind="ExternalInput")
    bq = nc.dram_tensor("bq", [P, NJ], f32, kind="ExternalInput")
    out_d = nc.dram_tensor("out_partial", [S, E], f16, kind="ExternalOutput")
    # DRAM staging for the fp8 partition regroup (128 -> 32x4)
    q_stage = nc.dram_tensor("q_stage", [NJ, P, S], f8, kind="Internal")
    k_stage = nc.dram_tensor("k_stage", [NJ, P, S], f8, kind="Internal")

    vw = DH + 1        # per-head V columns incl. the ones column

    with tile.TileContext(nc) as tc, ExitStack() as ctx:
        const = ctx.enter_context(tc.tile_pool(name="const", bufs=1))
        pt_pool = ctx.enter_context(tc.tile_pool(name="pt", bufs=32))
        cu_pool = ctx.enter_context(tc.tile_pool(name="cu", bufs=6))
        cq_pool = ctx.enter_context(tc.tile_pool(name="cq", bufs=6))
        outp = ctx.enter_context(tc.tile_pool(name="outp", bufs=6))
        ps_s = ctx.enter_context(tc.tile_pool(name="ps_s", bufs=2, space="PSUM"))
        ps_o = ctx.enter_context(tc.tile_pool(name="ps_o", bufs=2, space="PSUM"))
        ps_t = ctx.enter_context(tc.tile_pool(name="ps_t", bufs=1, space="PSUM"))
        # junk ramp-keeper matmuls write here: they keep the PE engine busy
        # through known feed gaps so the p-state never drops (matmul cost
        # is priced at dispatch with pe_busy_start reset on any engine
        # idle); single buf -> junk serializes on itself only
        ps_j = ctx.enter_context(tc.tile_pool(name="ps_j", bufs=1, space="PSUM"))

        # ---- resident SBUF tensors ----
        wq_sb = const.tile([P, NE, EH], f16)     # full W (v-proj + qk 2-7)
        wq8_sb = const.tile([P, NF8, EH], f8)
        wo_sb = const.tile([P, NJ, E], f16)      # [p, f-tile, o]
        bq_sb = const.tile([P, NJ], f32)
        junk = const.tile([P, 512], f16)
        ident = const.tile([P, P], f16)
        xq_sb = const.tile([P, NE16, S], f16)
        xk_sb = const.tile([P, NE16, S], f16)
        xq8_sb = const.tile([P, NF8, S], f8)
        xk8_sb = const.tile([P, NF8, S], f8)
        xv_sb = const.tile([P, NE, S], f16)
        # fp8 q/k staging (proj drain output, pre-regroup) [p=j, jt, s]
        q8_sb = const.tile([P, NJ - 1, S], f8)
        k8_sb = const.tile([P, NJ - 1, S], f8)
        # pair 0 (j-tile 0) keeps q/k in fp16 and runs fp16 scores straight
        # from the drain: no DRAM regroup round-trip on the critical path
        qt0_sb = const.tile([P, S], f16)
        kt0_sb = const.tile([P, S], f16)
        # DR-layout q/k [32, jt-1, head, dsub, s] (j-tiles 1..3 only;
        # pair 0 runs fp16)
        q8dr = const.tile([32, NJ - 1, 2, 2, S], f8)
        k8dr = const.tile([32, NJ - 1, 2, 2, S], f8)
        # V tiles [key-tile][p=k, 8 heads x (dh + ones col)]
        v_sb = [const.tile([P, NHG * vw], f16, tag=f"v{st}", name=f"v{st}")
                for st in range(NST)]
        # transposed attention output C^T [p=f, f-tile, s] fp16
        ct_sb = const.tile([P, NJ, S], f16)
        # out-proj partial accumulator [p=s, st, o] fp16
        oa_sb = const.tile([P, NST, S], f16)

        nc.sync.dma_start(out=bq_sb[:, :], in_=bq[:, :])
        nc.vector.memset(junk, 0.0)
        make_identity(nc, ident)
        wq_r = wq_t.rearrange("(t p) j -> p t j", p=P)
        wq8_r = wq8_t.rearrange("(t p) j -> p t j", p=P)
        xk_r = xk_t.rearrange("(t p) s -> p t s", p=P)
        xq_r = xq_t.rearrange("(t p) s -> p t s", p=P)
        xk8_r = xk8_t.rearrange("(t p) s -> p t s", p=P)
        xq8_r = xq8_t.rearrange("(t p) s -> p t s", p=P)
        xv_r = xv_t.rearrange("(t p) s -> p t s", p=P)
        wo_r = wo_t.rearrange("(t p) o -> p t o", p=P)
        # input DMA order tuned so the k/q projections can stream in
        # two-tile waves: fp8 parts first (tiny), then alternating
        # (xk, wq) two-tile transfers, then xq; wq e-tiles 0-1 (fp16,
        # only v-proj needs them), xv and wo stream last -- pair-0 scores
        # run fp16 straight from the drain so nothing critical queues
        # behind them in the serialized DMA-engine FIFO
        nc.sync.dma_start(out=xq8_sb[...], in_=xq8_r)
        nc.sync.dma_start(out=wq8_sb[...], in_=wq8_r)
        nc.sync.dma_start(out=xk8_sb[...], in_=xk8_r)
        for t in range(0, NE16, 2):
            nc.sync.dma_start(out=xq_sb[:, t:t + 2, :], in_=xq_r[:, t:t + 2, :])
            nc.sync.dma_start(out=wq_sb[:, t + NF8:t + NF8 + 2, :],
                              in_=wq_r[:, t + NF8:t + NF8 + 2, :])
        for t in range(0, NE16, 2):
            nc.sync.dma_start(out=xk_sb[:, t:t + 2, :], in_=xk_r[:, t:t + 2, :])
        nc.sync.dma_start(out=wq_sb[:, 0:NF8, :], in_=wq_r[:, 0:NF8, :])
        for h in range(2):
            sl = slice(h * 4, (h + 1) * 4)
            nc.sync.dma_start(out=xv_sb[:, sl, :], in_=xv_r[:, sl, :])
        nc.sync.dma_start(out=wo_sb[:, :, :], in_=wo_r)

        def body():
            def jmm(n, width=512):
                # junk ramp-keeper matmuls: ~213ns each of always-ready PE
                # work (they only read the zeroed junk tile)
                for _ in range(n):
                    jp = ps_j.tile([P, 512], f32, tag="junk", name="jp")
                    nc.tensor.matmul(jp[:, 0:width], lhsT=junk[:, 0:128],
                                     rhs=junk[:, 0:width],
                                     start=True, stop=True)

            # warm-up: bridge from t=0 until the first k-proj inputs land
            jmm(16)

            for st in range(NST):
                vh = v_sb[st].rearrange("p (h c) -> p h c", c=vw)
                nc.vector.memset(vh[:, :, DH], 1.0)

            # psum slots for projection-phase matmul groups: ps_o always;
            # ps_s only while it isn't busy with scores (head phase)
            rr_state = [0]

            def rr_psum(head=False):
                if head:
                    i = rr_state[0] % 4
                    rr_state[0] += 1
                    if i < 2:
                        t = ps_s.tile([P, S], f32, tag="ps_s", name="ps")
                        return t[:, 0:512]
                return ps_o.tile([P, 512], f32, tag="ps_o", name="ps")

            def proj_qk_group(jt, qc, x16, x8, dst8, bias, head=False):
                # one (jt, qc) projection group: 1 fp8-DR matmul over
                # e-tiles 0-1 + 6 fp16 matmuls over e-tiles 2-7
                ps = rr_psum(head)
                for dp in range(NF8 // 2):
                    nc.tensor.matmul(
                        ps,
                        lhsT=wq8_sb[:, 2 * dp:2 * dp + 2, jt * P:(jt + 1) * P],
                        rhs=x8[:, 2 * dp:2 * dp + 2, qc * 512:(qc + 1) * 512],
                        start=(dp == 0), stop=False,
                        perf_mode=DR,
                    )
                for t in range(NF8, NE):
                    nc.tensor.matmul(
                        ps,
                        lhsT=wq_sb[:, t, jt * P:(jt + 1) * P],
                        rhs=x16[:, t - NF8, qc * 512:(qc + 1) * 512],
                        start=False,
                        stop=(t == NE - 1),
                    )
                d = dst8[:, jt - 1, qc * 512:(qc + 1) * 512]
                if bias is not None:
                    nc.vector.tensor_scalar_add(d, ps, bias[:, jt:jt + 1])
                else:
                    nc.vector.tensor_copy(d, ps)

            def regroup(jt, src8, stage, dstdr):
                # SBUF -> DRAM -> SBUF partition fold 128 -> 32x(2 head,
                # 2 dsub); rides the gpsimd SWDGE queue so neither the SP
                # input stream nor the ACT exp queue serializes against it;
                # src8 holds j-tiles 1..3 at jt-1
                nc.gpsimd.dma_start(out=stage[jt], in_=src8[:, jt - 1, :])
                nc.gpsimd.dma_start(
                    out=dstdr[:, jt - 1, :, :, :],
                    in_=stage[jt].rearrange("(h d q) s -> q h d s", h=2, d=2),
                )

            def proj_v(st):
                ps = rr_psum()
                for t in range(NE):
                    nc.tensor.matmul(
                        ps,
                        lhsT=xv_sb[:, t, st * P:(st + 1) * P],
                        rhs=wq_sb[:, t, :],
                        start=(t == 0),
                        stop=(t == NE - 1),
                    )
                vh = v_sb[st].rearrange("p (h c) -> p h c", c=vw)
                nc.vector.tensor_copy(
                    vh[:, :, 0:DH],
                    ps.rearrange("p (h d) -> p h d", d=DH))

            pt_pairs = [[[], []] for _ in range(NJ)]

            def scores_exp_kt(jt, kt):
                # fp8 DoubleRow scores for the head pair of j-tile jt at
                # key-tile kt: per hh one [128, 1024] psum tile (2 banks)
                # covering both q-chunks; 2 DR matmuls fill it; exp is one
                # wide ACT op
                for hh in range(2):
                    pss = ps_s.tile([P, S], f32, tag="ps_s", name=f"pss{hh}")
                    pt = pt_pool.tile([P, S], f16, tag="pt", name=f"pt{hh}")
                    pt_pairs[jt][hh].append(pt)
                    bp = hh * DH
                    for qc in range(NQ):
                        if jt == 0:
                            nc.tensor.matmul(
                                pss[:, qc * 512:(qc + 1) * 512],
                                lhsT=kt0_sb[bp:bp + DH, kt * P:(kt + 1) * P],
                                rhs=qt0_sb[bp:bp + DH,
                                           qc * 512:(qc + 1) * 512],
                                start=True, stop=True,
                            )
                        else:
                            nc.tensor.matmul(
                                pss[:, qc * 512:(qc + 1) * 512],
                                lhsT=k8dr[:, jt - 1, hh, :,
                                          kt * P:(kt + 1) * P],
                                rhs=q8dr[:, jt - 1, hh, :,
                                         qc * 512:(qc + 1) * 512],
                                start=True, stop=True,
                                perf_mode=DR,
                            )
                    nc.scalar.activation(
                        out=pt, in_=pss, func=AF.Exp, scale=0.125,
                    )

            cq_store = {}

            def av_pair(jt, st):
                # AV for both heads of pair jt at query tile st, packed
                # into one psum bank: head hh occupies cols [hh*65,
                # hh*65+65) (64 dims + denominator from the V ones column)
                pts = pt_pairs[jt]
                po = ps_o.tile([P, 512], f32, tag="ps_o", name="po")
                for kt in range(NE):
                    for hh in range(2):
                        h = 2 * jt + hh
                        nc.tensor.matmul(
                            po[:, hh * vw:(hh + 1) * vw],
                            lhsT=pts[hh][kt][:, st * P:(st + 1) * P],
                            rhs=v_sb[kt][:, h * vw:(h + 1) * vw],
                            start=(kt == 0 and hh == 0),
                            stop=(kt == NE - 1 and hh == 1),
                            skip_group_check=True,
                        )
                cu = cu_pool.tile([P, 2 * vw], f32, tag="cu", name="cu")
                nc.vector.tensor_copy(cu, po[:, 0:2 * vw])
                cq = cq_pool.tile([P, 2, DH], f16, tag="cq", name="cq")
                for hh in range(2):
                    nc.gpsimd.normalize_recip(
                        cq[:, hh, :],
                        cu[:, hh * vw:hh * vw + DH],
                        cu[:, hh * vw + DH:(hh + 1) * vw],
                    )
                cq_store[(jt, st)] = cq

            def transpose_pair(jt, st):
                # [128 q, 64 d] per head -> psum [64 d, 128 q] stacked pair
                cq = cq_store.pop((jt, st))
                pst = ps_t.tile([P, P], f16, tag="ps_t", name="pst")
                for hh in range(2):
                    nc.tensor.matmul(
                        pst[hh * DH:(hh + 1) * DH, :],
                        lhsT=cq[:, hh, :],
                        rhs=ident,
                        is_transpose=True,
                        tile_position=(0, hh * DH),
                        skip_group_check=True,
                    )
                dst = ct_sb[:, jt, st * P:(st + 1) * P]
                if jt == NJ - 1:
                    # tail pair: ACT is free once the exp stream ends
                    nc.scalar.copy(dst, pst)
                else:
                    nc.vector.tensor_copy(dst, pst)

            def av_step(p, st):
                av_pair(p, st)
                if st >= 2:
                    transpose_pair(p, st - 2)
                if st == NST - 1:
                    transpose_pair(p, st - 1)
                    transpose_pair(p, st)

            def op_a(st, ocs=(0, 1)):
                # out-proj pass A: ft0+ft1+ft2 partial -> oa (fp16 staging)
                for oc in ocs:
                    ps = ps_o.tile([P, 512], f32, tag="ps_o", name="ps")
                    for ft in range(3):
                        nc.tensor.matmul(
                            ps,
                            lhsT=ct_sb[:, ft, st * P:(st + 1) * P],
                            rhs=wo_sb[:, ft, oc * 512:(oc + 1) * 512],
                            start=(ft == 0),
                            stop=(ft == 2),
                        )
                    nc.vector.tensor_copy(
                        oa_sb[:, st, oc * 512:(oc + 1) * 512], ps)

            def op_b(st, split_dma=False):
                # out-proj pass B (tail): ft3 + oa -> out, per-oc tiles and
                # stores (drains split across DVE and gpsimd)
                for oc in range(NQ):
                    sl = slice(oc * 512, (oc + 1) * 512)
                    ot = outp.tile([P, 512], f16, tag="ot", name="ot")
                    pst = ps_s.tile([P, S], f32, tag="ps_s", name="ps")
                    ps = pst[:, 0:512]
                    nc.tensor.matmul(
                        ps,
                        lhsT=ct_sb[:, 3, st * P:(st + 1) * P],
                        rhs=wo_sb[:, 3, sl],
                        start=True, stop=False,
                    )
                    nc.tensor.matmul(
                        ps,
                        lhsT=ident,
                        rhs=oa_sb[:, st, sl],
                        start=False, stop=True,
                    )
                    if oc == 0:
                        nc.vector.tensor_copy(ot, ps)
                    else:
                        # ACT is idle in the tail (exp stream done)
                        nc.scalar.copy(ot, ps)
                    nc.sync.dma_start(
                        out=out_d[st * P:(st + 1) * P, sl],
                        in_=ot,
                    )

            # ---- emission ----
            # head: k-proj then q-proj for j-tiles 0,1, tile-major with
            # junk bridges sized to the DMA arrival cadence (per-tile
            # transfer ~0.7us unlocks 4 matmuls ~0.85us; junk fills the
            # start-up slack).  JH tunables were set from the sim trace.
            JH = [6, 0, 0, 0, 0, 0]
            # q-proj first (xq now leads the load stream): j-tile 0 alone so
            # its fp16 drain lands earliest, then j-tile 1
            for jts in ((0,), (1,)):
                qg = [(jt, qc) for jt in jts for qc in range(NQ)]
                pss_q = {g: rr_psum(True) for g in qg}
                if jts == (0,):
                    jmm(JH[0])
                for dp in range(NF8 // 2):
                    for g in qg:
                        nc.tensor.matmul(
                            pss_q[g],
                            lhsT=wq8_sb[:, 2 * dp:2 * dp + 2,
                                        g[0] * P:(g[0] + 1) * P],
                            rhs=xq8_sb[:, 2 * dp:2 * dp + 2,
                                       g[1] * 512:(g[1] + 1) * 512],
                            start=(dp == 0), stop=False, perf_mode=DR)
                for t in range(NF8, NE):
                    for jt, qc in qg:
                        nc.tensor.matmul(
                            pss_q[(jt, qc)],
                            lhsT=wq_sb[:, t, jt * P:(jt + 1) * P],
                            rhs=xq_sb[:, t - NF8, qc * 512:(qc + 1) * 512],
                            start=False, stop=(t == NE - 1))
                for jt, qc in qg:
                    dst = (qt0_sb[:, qc * 512:(qc + 1) * 512] if jt == 0 else
                           q8_sb[:, jt - 1, qc * 512:(qc + 1) * 512])
                    nc.vector.tensor_scalar_add(dst, pss_q[(jt, qc)],
                                                bq_sb[:, jt:jt + 1])
            regroup(1, q8_sb, q_stage, q8dr)
            # k-proj: j-tile 0 wave + drain first (gates the exp stream)
            for jts in ((0,), (1,)):
                kg = [(jt, qc) for jt in jts for qc in range(NQ)]
                pss_k = {g: rr_psum(True) for g in kg}
                for dp in range(NF8 // 2):
                    for g in kg:
                        nc.tensor.matmul(
                            pss_k[g],
                            lhsT=wq8_sb[:, 2 * dp:2 * dp + 2,
                                        g[0] * P:(g[0] + 1) * P],
                            rhs=xk8_sb[:, 2 * dp:2 * dp + 2,
                                       g[1] * 512:(g[1] + 1) * 512],
                            start=(dp == 0), stop=False, perf_mode=DR)
                for t in range(NF8, NE):
                    for jt, qc in kg:
                        nc.tensor.matmul(
                            pss_k[(jt, qc)],
                            lhsT=wq_sb[:, t, jt * P:(jt + 1) * P],
                            rhs=xk_sb[:, t - NF8, qc * 512:(qc + 1) * 512],
                            start=False, stop=(t == NE - 1))
                for jt, qc in kg:
                    if jt == 0:
                        nc.vector.tensor_copy(
                            kt0_sb[:, qc * 512:(qc + 1) * 512],
                            pss_k[(jt, qc)])
                    else:
                        nc.vector.tensor_copy(
                            k8_sb[:, jt - 1, qc * 512:(qc + 1) * 512],
                            pss_k[(jt, qc)])
            regroup(1, k8_sb, k_stage, k8dr)

            # filler thunks woven between scores blocks
            def kq(jt, qc, x16, x8, dst8, bias):
                return lambda: proj_qk_group(jt, qc, x16, x8, dst8, bias)

            def rg(jt, src8, stage, dstdr):
                return lambda: regroup(jt, src8, stage, dstdr)

            def av(p, st):
                return lambda: av_step(p, st)

            def vp(st):
                return lambda: proj_v(st)

            def jk(n):
                return lambda: jmm(n)

            fillers = {
                # pair 0: j-tile-2 projections early (regroups well before
                # pair 2), then the first v-projections
                0: [[kq(2, 0, xk_sb, xk8_sb, k8_sb, None)],
                    [kq(2, 1, xk_sb, xk8_sb, k8_sb, None),
                     rg(2, k8_sb, k_stage, k8dr)],
                    [kq(2, 0, xq_sb, xq8_sb, q8_sb, bq_sb)],
                    [kq(2, 1, xq_sb, xq8_sb, q8_sb, bq_sb),
                     rg(2, q8_sb, q_stage, q8dr)],
                    [vp(0)], [vp(1)], [vp(2)], [vp(3)]],
                # pair 1: rest of v-proj, then AV(0)
                1: [[vp(4)], [vp(5)], [vp(6)], [vp(7)],
                    [av(0, 0), av(0, 1)],
                    [av(0, 2), av(0, 3), av(0, 4)],
                    [av(0, 5), av(0, 6)],
                    [av(0, 7), jk(3)]],
                # pair 2: j-tile-3 projections early + AV(1)
                2: [[kq(3, 0, xk_sb, xk8_sb, k8_sb, None), av(1, 0)],
                    [kq(3, 1, xk_sb, xk8_sb, k8_sb, None),
                     rg(3, k8_sb, k_stage, k8dr), av(1, 1)],
                    [kq(3, 0, xq_sb, xq8_sb, q8_sb, bq_sb), av(1, 2)],
                    [kq(3, 1, xq_sb, xq8_sb, q8_sb, bq_sb),
                     rg(3, q8_sb, q_stage, q8dr), av(1, 3)],
                    [av(1, 4), jk(4)],
                    [av(1, 5), jk(4)],
                    [av(1, 6), jk(4)],
                    [av(1, 7), jk(4)]],
                # pair 3: AV(2) first (pt ring), then out-proj pass A
                3: [[av(2, 0), av(2, 1), jk(2)],
                    [av(2, 2), av(2, 3), jk(1)],
                    [av(2, 4), av(2, 5), lambda: op_a(0, (0,))],
                    [av(2, 6), av(2, 7), lambda: op_a(0, (1,))],
                    [lambda: op_a(1), lambda: op_a(2, (0,))],
                    [lambda: op_a(2, (1,)), lambda: op_a(3)],
                    [lambda: op_a(4), lambda: op_a(5, (0,))],
                    [lambda: op_a(5, (1,)), lambda: op_a(6)]],
            }
            for p in range(NJ):
                for kt in range(NE):
                    scores_exp_kt(p, kt)
                    for th in fillers[p][kt]:
                        th()

            # tail: AV(3) -> transpose -> out-proj pass B chase (lag 2)
            op_a(7)
            for st in range(NST):
                av_pair(3, st)
                if st >= 1:
                    transpose_pair(3, st - 1)
                if st >= 2:
                    op_b(st - 2)
            transpose_pair(3, NST - 1)
            for st in range(NST - 2, NST):
                op_b(st)

        for _ in range(reps):
            body()

    nc.finalize()
    return nc


def _get_nc(reps=1):
    key = ("nc", reps)
    if key not in _CACHE:
        _CACHE[key] = _build_program(reps)
    return _CACHE[key]


def make_in_maps(queries, keys, values, Wq_w, Wq_b, Wo_w, Wo_b):
    in_maps = []
    cut = NF8 * P
    for c in range(NCORES):
        b, g = c // 2, c % 2
        js = slice(g * EH, (g + 1) * EH)
        qT = np.ascontiguousarray(queries[b].T)
        kT = np.ascontiguousarray(keys[b].T)
        wT = np.ascontiguousarray(Wq_w[js, :].T)
        in_maps.append({
            "xq_t": qT[cut:].astype(F16),
            "xk_t": kT[cut:].astype(F16),
            "xv_t": np.ascontiguousarray(values[b].T).astype(F16),
            "xq8_t": qT[:cut].astype(F8),
            "xk8_t": kT[:cut].astype(F8),
            "wq_t": wT.astype(F16),
            "wq8_t": wT[:cut].astype(F8),
            "wo_t": np.ascontiguousarray(Wo_w[:, js].T).astype(F16),
            "bq": np.ascontiguousarray(Wq_b[js].reshape(NJ, P).T),
        })
    return in_maps


def assemble_output(results, Wq_b, Wo_w, Wo_b):
    bias_total = (Wo_w @ Wq_b + Wo_b).astype(np.float32)
    out = np.empty((B, S, E), np.float32)
    for b in range(B):
        out[b] = (results[2 * b]["out_partial"].astype(np.float32)
                  + results[2 * b + 1]["out_partial"].astype(np.float32))
    out += bias_total
    return out


def kernel(queries, keys, values, Wq_w, Wq_b, Wo_w, Wo_b, num_heads):
    from concourse.bass_utils import run_bass_kernel_spmd

    queries = np.asarray(queries, np.float32)
    keys = np.asarray(keys, np.float32)
    values = np.asarray(values, np.float32)
    Wq_w = np.asarray(Wq_w, np.float32)
    Wq_b = np.asarray(Wq_b, np.float32)
    Wo_w = np.asarray(Wo_w, np.float32)
    Wo_b = np.asarray(Wo_b, np.float32)
    assert int(num_heads) == H

    nc = _get_nc()
    in_maps = make_in_maps(queries, keys, values, Wq_w, Wq_b, Wo_w, Wo_b)
    res = run_bass_kernel_spmd(nc, in_maps, core_ids=list(range(NCORES)))
    _CACHE["last_results"] = res
    return assemble_output(res.results, Wq_b, Wo_w, Wo_b)


# revision 7
# speedup vs baseline: 1.0670x; 1.0050x over previous
"""Trainium2 Bass kernel for nn_MultiHeadAttention_79534204387726.

Reference computation (B=4, S=1024, E=1024, H=16, dh=64):
    q/k/v = proj(x) = x @ Wq_w.T + Wq_b       (same Wq applied to q, k, v)
    scores = q @ k.T / 8 per head; attn = softmax(scores)
    out = (attn @ v).concat_heads @ Wo_w.T + Wo_b

Sharding (8 cores): core c -> batch b = c//2, head-group g = c%2 (8 heads,
512 features). Host sums the two partial outputs per batch and adds the
folded bias (Wo_b + Wo@Wq_b; K-bias is softmax-invariant and dropped).

v2 design (cost-model driven, TimelineSim):
  - The kernel is paced by the ACT exp stream (64 x [128,1024] exp tiles =
    66.4us, the single-engine floor).  Emission weaves ~1.65us of filler PE
    work between each scores(jt, kt) block so the PE (89us of work) stays
    busy under the exp cadence: v-proj and j-tile-2/3 projections fill the
    early pairs, AV + transposes of the previous pair and early out-proj
    passes fill the later ones.
  - scores matmuls run in fp8e4m3 DoubleRow (0.5 cycles/row): q/k are
    quantized to fp8 during the psum drain (q with bias add), staged to
    DRAM, and reloaded as [32, head, dsub, S] (the 128->32 partition fold
    needs a DMA round trip).  Those DMAs ride the ACT DGE queue so they
    don't queue behind the input loads on SP.
  - The first half of the q/k projection contraction (e-tiles 0-3) also
    runs in fp8 DoubleRow from host-quantized x8/wq8.  Measured end-to-end
    rel_err 0.0169 (gate 2e-2; pair 0 stays fp16 which also skips the
    regroup round-trip on the exp-stream critical path).
  - AV is flipped to out[q, d]: lhsT = exp(scores^T) [k, q] chunk, rhs = V
    [k, 65] per head (64 dims + ones column -> denominator lands in psum
    col 64 per q row): 65 free-cols/instruction instead of 512.  Two heads
    pack into one psum bank (second head's first write exploits the lazy
    2KB zero-region).
  - softmax normalize runs on gpsimd (normalize_recip: divide + fp16 cast).
  - C comes out as [q, d]; a PE transpose (identity matmul) restores the
    [feature, seq] layout the out-projection needs as lhsT.
  - out-proj is split: ft0+ft1 partials during pair 2, += ft2 during pair
    3 (fp16 SBUF staging, identity-matmul re-accumulation), ft3 + final
    add in the tail so only ~1.4us/st of PE work remains after the last
    exp.
"""

import numpy as np
import ml_dtypes

B, S, E, H = 4, 1024, 1024, 16
NCORES = 8
EH = E // 2        # 512 features per head-group
NHG = H // 2       # 8 heads per group
DH = E // H        # 64
P = 128
NE = E // P        # 8 e-tiles over full E
NF8 = 4            # e-tiles 0-3 of the q/k projection contract in fp8 DR
NE16 = NE - NF8    # e-tiles 2-7 in fp16
NJ = EH // P       # 4 j-tiles over the group's 512 features
NQ = S // 512      # 2 query/sequence chunks of 512
NST = S // P       # 8 sequence tiles of 128
F16 = np.float16
F8 = ml_dtypes.float8_e4m3fn

_CACHE = {}


def _build_program(reps=1):
    import concourse.tile as tile
    from concourse import bacc, mybir
    from concourse.masks import make_identity
    from contextlib import ExitStack

    f32 = mybir.dt.float32
    f16 = mybir.dt.float16
    f8 = mybir.dt.float8e4
    AF = mybir.ActivationFunctionType
    DR = mybir.MatmulPerfMode.DoubleRow

    nc = bacc.Bacc(
        "TRN2",
        target_bir_lowering=False,
        debug=False,
        num_devices=NCORES,
    )

    # fp16 x for q/k carries only e-tiles 2-7; tiles 0-1 come as fp8
    xq_t = nc.dram_tensor("xq_t", [NE16 * P, S], f16, kind="ExternalInput")
    xk_t = nc.dram_tensor("xk_t", [NE16 * P, S], f16, kind="ExternalInput")
    xv_t = nc.dram_tensor("xv_t", [E, S], f16, kind="ExternalInput")
    xq8_t = nc.dram_tensor("xq8_t", [NF8 * P, S], f8, kind="ExternalInput")
    xk8_t = nc.dram_tensor("xk8_t", [NF8 * P, S], f8, kind="ExternalInput")
    wq_t = nc.dram_tensor("wq_t", [E, EH], f16, kind="ExternalInput")
    wq8_t = nc.dram_tensor("wq8_t", [NF8 * P, EH], f8, kind="ExternalInput")
    wo_t = nc.dram_tensor("wo_t", [EH, E], f16, kind="ExternalInput")
    bq = nc.dram_tensor("bq", [P, NJ], f32, kind="ExternalInput")
    out_d = nc.dram_tensor("out_partial", [S, E], f16, kind="ExternalOutput")
    # DRAM staging for the fp8 partition regroup (128 -> 32x4)
    q_stage = nc.dram_tensor("q_stage", [NJ, P, S], f8, kind="Internal")
    k_stage = nc.dram_tensor("k_stage", [NJ, P, S], f8, kind="Internal")

    vw = DH + 1        # per-head V columns incl. the ones column

    with tile.TileContext(nc) as tc, ExitStack() as ctx:
        const = ctx.enter_context(tc.tile_pool(name="const", bufs=1))
        pt_pool = ctx.enter_context(tc.tile_pool(name="pt", bufs=32))
        cu_pool = ctx.enter_context(tc.tile_pool(name="cu", bufs=20))
        cq_pool = ctx.enter_context(tc.tile_pool(name="cq", bufs=6))
        outp = ctx.enter_context(tc.tile_pool(name="outp", bufs=6))
        ps_s = ctx.enter_context(tc.tile_pool(name="ps_s", bufs=2, space="PSUM"))
        ps_o = ctx.enter_context(tc.tile_pool(name="ps_o", bufs=2, space="PSUM"))
        ps_t = ctx.enter_context(tc.tile_pool(name="ps_t", bufs=1, space="PSUM"))
        # junk ramp-keeper matmuls write here: they keep the PE engine busy
        # through known feed gaps so the p-state never drops (matmul cost
        # is priced at dispatch with pe_busy_start reset on any engine
        # idle); single buf -> junk serializes on itself only
        ps_j = ctx.enter_context(tc.tile_pool(name="ps_j", bufs=1, space="PSUM"))

        # ---- resident SBUF tensors ----
        wq_sb = const.tile([P, NE, EH], f16)     # full W (v-proj + qk 2-7)
        wq8_sb = const.tile([P, NF8, EH], f8)
        wo_sb = const.tile([P, NJ, E], f16)      # [p, f-tile, o]
        bq_sb = const.tile([P, NJ], f32)
        junk = const.tile([P, 512], f16)
        ident = const.tile([P, P], f16)
        xq_sb = const.tile([P, NE16, S], f16)
        xk_sb = const.tile([P, NE16, S], f16)
        xq8_sb = const.tile([P, NF8, S], f8)
        xk8_sb = const.tile([P, NF8, S], f8)
        xv_sb = const.tile([P, NE, S], f16)
        # fp8 q/k staging (proj drain output, pre-regroup) [p=j, jt, s]
        q8_sb = const.tile([P, NJ - 1, S], f8)
        k8_sb = const.tile([P, NJ - 1, S], f8)
        # pair 0 (j-tile 0) keeps q/k in fp16 and runs fp16 scores straight
        # from the drain: no DRAM regroup round-trip on the critical path
        qt0_sb = const.tile([P, S], f16)
        kt0_sb = const.tile([P, S], f16)
        # DR-layout q/k [32, jt-1, head, dsub, s] (j-tiles 1..3 only;
        # pair 0 runs fp16)
        q8dr = const.tile([32, NJ - 1, 2, 2, S], f8)
        k8dr = const.tile([32, NJ - 1, 2, 2, S], f8)
        # V tiles [key-tile][p=k, 8 heads x (dh + ones col)]
        v_sb = [const.tile([P, NHG * vw], f16, tag=f"v{st}", name=f"v{st}")
                for st in range(NST)]
        # transposed attention output C^T [p=f, f-tile, s] fp16
        ct_sb = const.tile([P, NJ, S], f16)
        # out-proj partial accumulator [p=s, st, o] fp16
        oa_sb = const.tile([P, NST, S], f16)

        nc.sync.dma_start(out=bq_sb[:, :], in_=bq[:, :])
        nc.vector.memset(junk, 0.0)
        make_identity(nc, ident)
        wq_r = wq_t.rearrange("(t p) j -> p t j", p=P)
        wq8_r = wq8_t.rearrange("(t p) j -> p t j", p=P)
        xk_r = xk_t.rearrange("(t p) s -> p t s", p=P)
        xq_r = xq_t.rearrange("(t p) s -> p t s", p=P)
        xk8_r = xk8_t.rearrange("(t p) s -> p t s", p=P)
        xq8_r = xq8_t.rearrange("(t p) s -> p t s", p=P)
        xv_r = xv_t.rearrange("(t p) s -> p t s", p=P)
        wo_r = wo_t.rearrange("(t p) o -> p t o", p=P)
        # input DMA order tuned so the k/q projections can stream in
        # two-tile waves: fp8 parts first (tiny), then alternating
        # (xk, wq) two-tile transfers, then xq; wq e-tiles 0-1 (fp16,
        # only v-proj needs them), xv and wo stream last -- pair-0 scores
        # run fp16 straight from the drain so nothing critical queues
        # behind them in the serialized DMA-engine FIFO
        nc.sync.dma_start(out=xq8_sb[...], in_=xq8_r)
        nc.sync.dma_start(out=wq8_sb[...], in_=wq8_r)
        nc.sync.dma_start(out=xk8_sb[...], in_=xk8_r)
        for t in range(0, NE16, 2):
            nc.sync.dma_start(out=xq_sb[:, t:t + 2, :], in_=xq_r[:, t:t + 2, :])
            nc.sync.dma_start(out=wq_sb[:, t + NF8:t + NF8 + 2, :],
                              in_=wq_r[:, t + NF8:t + NF8 + 2, :])
        for t in range(0, NE16, 2):
            nc.sync.dma_start(out=xk_sb[:, t:t + 2, :], in_=xk_r[:, t:t + 2, :])
        nc.sync.dma_start(out=wq_sb[:, 0:NF8, :], in_=wq_r[:, 0:NF8, :])
        for h in range(2):
            sl = slice(h * 4, (h + 1) * 4)
            nc.sync.dma_start(out=xv_sb[:, sl, :], in_=xv_r[:, sl, :])
        nc.sync.dma_start(out=wo_sb[:, :, :], in_=wo_r)

        def body():
            def jmm(n, width=512):
                # junk ramp-keeper matmuls: ~213ns each of always-ready PE
                # work (they only read the zeroed junk tile)
                for _ in range(n):
                    jp = ps_j.tile([P, 512], f32, tag="junk", name="jp")
                    nc.tensor.matmul(jp[:, 0:width], lhsT=junk[:, 0:128],
                                     rhs=junk[:, 0:width],
                                     start=True, stop=True)

            # warm-up: bridge from t=0 until the first k-proj inputs land
            jmm(16)

            for st in range(NST):
                vh = v_sb[st].rearrange("p (h c) -> p h c", c=vw)
                nc.vector.memset(vh[:, :, DH], 1.0)

            # psum slots for projection-phase matmul groups: ps_o always;
            # ps_s only while it isn't busy with scores (head phase)
            rr_state = [0]

            def rr_psum(head=False):
                if head:
                    i = rr_state[0] % 4
                    rr_state[0] += 1
                    if i < 2:
                        t = ps_s.tile([P, S], f32, tag="ps_s", name="ps")
                        return t[:, 0:512]
                return ps_o.tile([P, 512], f32, tag="ps_o", name="ps")

            def proj_qk_group(jt, qc, x16, x8, dst8, bias, head=False):
                # one (jt, qc) projection group: 1 fp8-DR matmul over
                # e-tiles 0-1 + 6 fp16 matmuls over e-tiles 2-7
                ps = rr_psum(head)
                for dp in range(NF8 // 2):
                    nc.tensor.matmul(
                        ps,
                        lhsT=wq8_sb[:, 2 * dp:2 * dp + 2, jt * P:(jt + 1) * P],
                        rhs=x8[:, 2 * dp:2 * dp + 2, qc * 512:(qc + 1) * 512],
                        start=(dp == 0), stop=False,
                        perf_mode=DR,
                    )
                for t in range(NF8, NE):
                    nc.tensor.matmul(
                        ps,
                        lhsT=wq_sb[:, t, jt * P:(jt + 1) * P],
                        rhs=x16[:, t - NF8, qc * 512:(qc + 1) * 512],
                        start=False,
                        stop=(t == NE - 1),
                    )
                d = dst8[:, jt - 1, qc * 512:(qc + 1) * 512]
                if bias is not None:
                    nc.vector.tensor_scalar_add(d, ps, bias[:, jt:jt + 1])
                else:
                    nc.vector.tensor_copy(d, ps)

            def regroup(jt, src8, stage, dstdr):
                # SBUF -> DRAM -> SBUF partition fold 128 -> 32x(2 head,
                # 2 dsub); rides the gpsimd SWDGE queue so neither the SP
                # input stream nor the ACT exp queue serializes against it;
                # src8 holds j-tiles 1..3 at jt-1
                nc.gpsimd.dma_start(out=stage[jt], in_=src8[:, jt - 1, :])
                nc.gpsimd.dma_start(
                    out=dstdr[:, jt - 1, :, :, :],
                    in_=stage[jt].rearrange("(h d q) s -> q h d s", h=2, d=2),
                )

            def proj_v(st):
                ps = rr_psum()
                for t in range(NE):
                    nc.tensor.matmul(
                        ps,
                        lhsT=xv_sb[:, t, st * P:(st + 1) * P],
                        rhs=wq_sb[:, t, :],
                        start=(t == 0),
                        stop=(t == NE - 1),
                    )
                vh = v_sb[st].rearrange("p (h c) -> p h c", c=vw)
                nc.vector.tensor_copy(
                    vh[:, :, 0:DH],
                    ps.rearrange("p (h d) -> p h d", d=DH))

            pt_pairs = [[[], []] for _ in range(NJ)]

            def scores_exp_kt(jt, kt):
                # fp8 DoubleRow scores for the head pair of j-tile jt at
                # key-tile kt: per hh one [128, 1024] psum tile (2 banks)
                # covering both q-chunks; 2 DR matmuls fill it; exp is one
                # wide ACT op
                for hh in range(2):
                    pss = ps_s.tile([P, S], f32, tag="ps_s", name=f"pss{hh}")
                    pt = pt_pool.tile([P, S], f16, tag="pt", name=f"pt{hh}")
                    pt_pairs[jt][hh].append(pt)
                    bp = hh * DH
                    for qc in range(NQ):
                        if jt == 0:
                            nc.tensor.matmul(
                                pss[:, qc * 512:(qc + 1) * 512],
                                lhsT=kt0_sb[bp:bp + DH, kt * P:(kt + 1) * P],
                                rhs=qt0_sb[bp:bp + DH,
                                           qc * 512:(qc + 1) * 512],
                                start=True, stop=True,
                            )
                        else:
                            nc.tensor.matmul(
                                pss[:, qc * 512:(qc + 1) * 512],
                                lhsT=k8dr[:, jt - 1, hh, :,
                                          kt * P:(kt + 1) * P],
                                rhs=q8dr[:, jt - 1, hh, :,
                                         qc * 512:(qc + 1) * 512],
                                start=True, stop=True,
                                perf_mode=DR,
                            )
                    nc.scalar.activation(
                        out=pt, in_=pss, func=AF.Exp, scale=0.125,
                    )

            cq_store = {}

            def av_pair(jt, st):
                # AV for both heads of pair jt at query tile st, packed
                # into one psum bank: head hh occupies cols [hh*65,
                # hh*65+65) (64 dims + denominator from the V ones column)
                pts = pt_pairs[jt]
                po = ps_o.tile([P, 512], f32, tag="ps_o", name="po")
                for kt in range(NE):
                    for hh in range(2):
                        h = 2 * jt + hh
                        nc.tensor.matmul(
                            po[:, hh * vw:(hh + 1) * vw],
                            lhsT=pts[hh][kt][:, st * P:(st + 1) * P],
                            rhs=v_sb[kt][:, h * vw:(h + 1) * vw],
                            start=(kt == 0 and hh == 0),
                            stop=(kt == NE - 1 and hh == 1),
                            skip_group_check=True,
                        )
                cu = cu_pool.tile([P, 2 * vw], f32, tag="cu", name="cu")
                nc.vector.tensor_copy(cu, po[:, 0:2 * vw])
                cq = cq_pool.tile([P, 2, DH], f16, tag="cq", name="cq")
                for hh in range(2):
                    nc.gpsimd.normalize_recip(
                        cq[:, hh, :],
                        cu[:, hh * vw:hh * vw + DH],
                        cu[:, hh * vw + DH:(hh + 1) * vw],
                    )
                cq_store[(jt, st)] = cq

            def transpose_pair(jt, st):
                # [128 q, 64 d] per head -> psum [64 d, 128 q] stacked pair
                cq = cq_store.pop((jt, st))
                pst = ps_t.tile([P, P], f16, tag="ps_t", name="pst")
                for hh in range(2):
                    nc.tensor.matmul(
                        pst[hh * DH:(hh + 1) * DH, :],
                        lhsT=cq[:, hh, :],
                        rhs=ident,
                        is_transpose=True,
                        tile_position=(0, hh * DH),
                        skip_group_check=True,
                    )
                dst = ct_sb[:, jt, st * P:(st + 1) * P]
                if jt == NJ - 1:
                    # tail pair: ACT is free once the exp stream ends
                    nc.scalar.copy(dst, pst)
                else:
                    nc.vector.tensor_copy(dst, pst)

            def av_step(p, st):
                av_pair(p, st)
                if st >= 2:
                    transpose_pair(p, st - 2)
                if st == NST - 1:
                    transpose_pair(p, st - 1)
                    transpose_pair(p, st)

            def op_a(st, ocs=(0, 1)):
                # out-proj pass A: ft0+ft1+ft2 partial -> oa (fp16 staging)
                for oc in ocs:
                    ps = ps_o.tile([P, 512], f32, tag="ps_o", name="ps")
                    for ft in range(3):
                        nc.tensor.matmul(
                            ps,
                            lhsT=ct_sb[:, ft, st * P:(st + 1) * P],
                            rhs=wo_sb[:, ft, oc * 512:(oc + 1) * 512],
                            start=(ft == 0),
                            stop=(ft == 2),
                        )
                    nc.vector.tensor_copy(
                        oa_sb[:, st, oc * 512:(oc + 1) * 512], ps)

            def op_b(st, split_dma=False):
                # out-proj pass B (tail): ft3 + oa -> out, per-oc tiles and
                # stores (drains split across DVE and gpsimd)
                for oc in range(NQ):
                    sl = slice(oc * 512, (oc + 1) * 512)
                    ot = outp.tile([P, 512], f16, tag="ot", name="ot")
                    pst = ps_s.tile([P, S], f32, tag="ps_s", name="ps")
                    ps = pst[:, 0:512]
                    nc.tensor.matmul(
                        ps,
                        lhsT=ct_sb[:, 3, st * P:(st + 1) * P],
                        rhs=wo_sb[:, 3, sl],
                        start=True, stop=False,
                    )
                    nc.tensor.matmul(
                        ps,
                        lhsT=ident,
                        rhs=oa_sb[:, st, sl],
                        start=False, stop=True,
                    )
                    if oc == 0:
                        nc.vector.tensor_copy(ot, ps)
                    else:
                        # ACT is idle in the tail (exp stream done)
                        nc.scalar.copy(ot, ps)
                    nc.sync.dma_start(
                        out=out_d[st * P:(st + 1) * P, sl],
                        in_=ot,
                    )

            # ---- emission ----
            # head: k-proj then q-proj for j-tiles 0,1, tile-major with
            # junk bridges sized to the DMA arrival cadence (per-tile
            # transfer ~0.7us unlocks 4 matmuls ~0.85us; junk fills the
            # start-up slack).  JH tunables were set from the sim trace.
            JH = [6, 0, 0, 0, 0, 0]
            # q-proj first (xq now leads the load stream): j-tile 0 alone so
            # its fp16 drain lands earliest, then j-tile 1
            for jts in ((0,), (1,)):
                qg = [(jt, qc) for jt in jts for qc in range(NQ)]
                pss_q = {g: rr_psum(True) for g in qg}
                if jts == (0,):
                    jmm(JH[0])
                for dp in range(NF8 // 2):
                    for g in qg:
                        nc.tensor.matmul(
                            pss_q[g],
                            lhsT=wq8_sb[:, 2 * dp:2 * dp + 2,
                                        g[0] * P:(g[0] + 1) * P],
                            rhs=xq8_sb[:, 2 * dp:2 * dp + 2,
                                       g[1] * 512:(g[1] + 1) * 512],
                            start=(dp == 0), stop=False, perf_mode=DR)
                for t in range(NF8, NE):
                    for jt, qc in qg:
                        nc.tensor.matmul(
                            pss_q[(jt, qc)],
                            lhsT=wq_sb[:, t, jt * P:(jt + 1) * P],
                            rhs=xq_sb[:, t - NF8, qc * 512:(qc + 1) * 512],
                            start=False, stop=(t == NE - 1))
                for jt, qc in qg:
                    dst = (qt0_sb[:, qc * 512:(qc + 1) * 512] if jt == 0 else
                           q8_sb[:, jt - 1, qc * 512:(qc + 1) * 512])
                    nc.vector.tensor_scalar_add(dst, pss_q[(jt, qc)],
                                                bq_sb[:, jt:jt + 1])
            regroup(1, q8_sb, q_stage, q8dr)
            # k-proj: j-tile 0 wave + drain first (gates the exp stream)
            for jts in ((0,), (1,)):
                kg = [(jt, qc) for jt in jts for qc in range(NQ)]
                pss_k = {g: rr_psum(True) for g in kg}
                for dp in range(NF8 // 2):
                    for g in kg:
                        nc.tensor.matmul(
                            pss_k[g],
                            lhsT=wq8_sb[:, 2 * dp:2 * dp + 2,
                                        g[0] * P:(g[0] + 1) * P],
                            rhs=xk8_sb[:, 2 * dp:2 * dp + 2,
                                       g[1] * 512:(g[1] + 1) * 512],
                            start=(dp == 0), stop=False, perf_mode=DR)
                for t in range(NF8, NE):
                    for jt, qc in kg:
                        nc.tensor.matmul(
                            pss_k[(jt, qc)],
                            lhsT=wq_sb[:, t, jt * P:(jt + 1) * P],
                            rhs=xk_sb[:, t - NF8, qc * 512:(qc + 1) * 512],
                            start=False, stop=(t == NE - 1))
                for jt, qc in kg:
                    if jt == 0:
                        nc.vector.tensor_copy(
                            kt0_sb[:, qc * 512:(qc + 1) * 512],
                            pss_k[(jt, qc)])
                    else:
                        nc.vector.tensor_copy(
                            k8_sb[:, jt - 1, qc * 512:(qc + 1) * 512],
                            pss_k[(jt, qc)])
            regroup(1, k8_sb, k_stage, k8dr)

            # filler thunks woven between scores blocks
            def kq(jt, qc, x16, x8, dst8, bias):
                return lambda: proj_qk_group(jt, qc, x16, x8, dst8, bias)

            def rg(jt, src8, stage, dstdr):
                return lambda: regroup(jt, src8, stage, dstdr)

            def av(p, st):
                return lambda: av_step(p, st)

            def vp(st):
                return lambda: proj_v(st)

            def jk(n):
                return lambda: jmm(n)

            fillers = {
                # pair 0: j-tile-2 projections early (regroups well before
                # pair 2), then the first v-projections
                0: [[kq(2, 0, xk_sb, xk8_sb, k8_sb, None)],
                    [kq(2, 1, xk_sb, xk8_sb, k8_sb, None),
                     rg(2, k8_sb, k_stage, k8dr)],
                    [kq(2, 0, xq_sb, xq8_sb, q8_sb, bq_sb)],
                    [kq(2, 1, xq_sb, xq8_sb, q8_sb, bq_sb),
                     rg(2, q8_sb, q_stage, q8dr)],
                    [vp(0)], [vp(1)], [vp(2)], [vp(3)]],
                # pair 1: rest of v-proj, then AV(0)
                1: [[vp(4)], [vp(5)], [vp(6)], [vp(7)],
                    [av(0, 0), av(0, 1)],
                    [av(0, 2), av(0, 3), av(0, 4)],
                    [av(0, 5), av(0, 6)],
                    [av(0, 7), jk(3)]],
                # pair 2: j-tile-3 projections early + AV(1)
                2: [[kq(3, 0, xk_sb, xk8_sb, k8_sb, None), av(1, 0)],
                    [kq(3, 1, xk_sb, xk8_sb, k8_sb, None),
                     rg(3, k8_sb, k_stage, k8dr), av(1, 1)],
                    [kq(3, 0, xq_sb, xq8_sb, q8_sb, bq_sb), av(1, 2)],
                    [kq(3, 1, xq_sb, xq8_sb, q8_sb, bq_sb),
                     rg(3, q8_sb, q_stage, q8dr), av(1, 3)],
                    [av(1, 4), jk(4)],
                    [av(1, 5), jk(4)],
                    [av(1, 6), jk(4)],
                    [av(1, 7), jk(4)]],
                # pair 3: AV(2) first (pt ring), then out-proj pass A
                3: [[av(2, 0), av(2, 1), jk(2)],
                    [av(2, 2), av(2, 3), jk(1)],
                    [av(2, 4), av(2, 5), lambda: op_a(0, (0,))],
                    [av(2, 6), av(2, 7), lambda: op_a(0, (1,))],
                    [lambda: op_a(1), lambda: op_a(2, (0,))],
                    [lambda: op_a(2, (1,)), lambda: op_a(3)],
                    [lambda: op_a(4), lambda: op_a(5, (0,))],
                    [lambda: op_a(5, (1,)), lambda: op_a(6)]],
            }
            for p in range(NJ):
                for kt in range(NE):
                    scores_exp_kt(p, kt)
                    for th in fillers[p][kt]:
                        th()

            # tail: AV(3) -> transpose -> out-proj pass B chase (lag 2)
            op_a(7)
            for st in range(NST):
                av_pair(3, st)
                if st >= 1:
                    transpose_pair(3, st - 1)
                if st >= 2:
                    op_b(st - 2)
            transpose_pair(3, NST - 1)
            for st in range(NST - 2, NST):
                op_b(st)

        for _ in range(reps):
            body()

    nc.finalize()
    return nc


def _get_nc(reps=1):
    key = ("nc", reps)
    if key not in _CACHE:
        _CACHE[key] = _build_program(reps)
    return _CACHE[key]


def make_in_maps(queries, keys, values, Wq_w, Wq_b, Wo_w, Wo_b):
    in_maps = []
    cut = NF8 * P
    for c in range(NCORES):
        b, g = c // 2, c % 2
        js = slice(g * EH, (g + 1) * EH)
        qT = np.ascontiguousarray(queries[b].T)
        kT = np.ascontiguousarray(keys[b].T)
        wT = np.ascontiguousarray(Wq_w[js, :].T)
        in_maps.append({
            "xq_t": qT[cut:].astype(F16),
            "xk_t": kT[cut:].astype(F16),
            "xv_t": np.ascontiguousarray(values[b].T).astype(F16),
            "xq8_t": qT[:cut].astype(F8),
            "xk8_t": kT[:cut].astype(F8),
            "wq_t": wT.astype(F16),
            "wq8_t": wT[:cut].astype(F8),
            "wo_t": np.ascontiguousarray(Wo_w[:, js].T).astype(F16),
            "bq": np.ascontiguousarray(Wq_b[js].reshape(NJ, P).T),
        })
    return in_maps


def assemble_output(results, Wq_b, Wo_w, Wo_b):
    bias_total = (Wo_w @ Wq_b + Wo_b).astype(np.float32)
    out = np.empty((B, S, E), np.float32)
    for b in range(B):
        out[b] = (results[2 * b]["out_partial"].astype(np.float32)
                  + results[2 * b + 1]["out_partial"].astype(np.float32))
    out += bias_total
    return out


def kernel(queries, keys, values, Wq_w, Wq_b, Wo_w, Wo_b, num_heads):
    from concourse.bass_utils import run_bass_kernel_spmd

    queries = np.asarray(queries, np.float32)
    keys = np.asarray(keys, np.float32)
    values = np.asarray(values, np.float32)
    Wq_w = np.asarray(Wq_w, np.float32)
    Wq_b = np.asarray(Wq_b, np.float32)
    Wo_w = np.asarray(Wo_w, np.float32)
    Wo_b = np.asarray(Wo_b, np.float32)
    assert int(num_heads) == H

    nc = _get_nc()
    in_maps = make_in_maps(queries, keys, values, Wq_w, Wq_b, Wo_w, Wo_b)
    res = run_bass_kernel_spmd(nc, in_maps, core_ids=list(range(NCORES)))
    _CACHE["last_results"] = res
    return assemble_output(res.results, Wq_b, Wo_w, Wo_b)
